# revision 1
# baseline (speedup 1.0000x reference)
"""DiffusionNet forward on 8 Trainium2 NeuronCores.

Strategy
--------
B=4 samples, 2 cores per sample, each core owns half the mesh nodes
(20000, zero-padded to 20480).  All cross-node coupling flows through the
K=128 spectral bottleneck:

  * SpMM is eliminated on-device: gX = G @ x_diffuse = (G @ evecs) @ S with
    S = coefs * x_spec, so host precomputes GXe = G @ evecs once per sample
    (exact associativity; measured 4e-7 rel err).
  * Per block: partial x_spec^T = sum_n x[n,:]^T evm[n,:] over owned nodes
    (PE accumulation), pairwise AllReduce of the [C,K] partial (64KB), then a
    fused channel-major sweep over node chunks computes x_diffuse, gX, gY,
    Breal, Bimag, grad_feat, the MLP and the residual without touching HBM
    for intermediates.

Layouts: per-node tensors live channel-major ([C, n]) in SBUF; x carried in
fp32 (+ a bf16 shadow for matmul operands), streamed operands in bf16.
"""

import sys
import numpy as np
import ml_dtypes

for _p in ("/opt/trn_rl_repo", "/root/.axon_site/_ro/trn_rl_repo"):
    if _p not in sys.path:
        sys.path.append(_p)

import concourse.bass as bass
import concourse.bacc as bacc
import concourse.tile as tile
import concourse.mybir as mybir
from concourse.bass_utils import run_bass_kernel_spmd
from concourse.masks import make_identity

BF = mybir.dt.bfloat16
F32 = mybir.dt.float32
F32R = mybir.dt.float32r
AF = mybir.ActivationFunctionType
ALU = mybir.AluOpType

B, N, E, K = 4, 40000, 240000, 128
C = 128
NB = 4          # diffusion blocks
NCORES = 8
NH = N // 2     # nodes per core (half sample)
CH = 512        # node chunk (matmul free dim)
NHP = 20480     # padded nodes per core: 40 chunks * 512 = 160 tiles * 128
NCH = NHP // CH
NT = NHP // 128
PAIRS = [[0, 1], [2, 3], [4, 5], [6, 7]]

bf16 = ml_dtypes.bfloat16


# ----------------------------------------------------------------- host side

def _spmm_mat(rows, cols, vals, M):
    """(COO [N,N] with given pattern) @ M, dense M [N,k]. Pure numpy."""
    out = np.zeros((N, M.shape[1]), np.float32)
    perm = np.argsort(rows, kind="stable")
    contrib = (vals[:, None] * M[cols]).astype(np.float32)[perm]
    rs = rows[perm]
    uniq, starts = np.unique(rs, return_index=True)
    out[uniq] = np.add.reduceat(contrib, starts, axis=0)
    return out


def host_prep(inputs, nhp=NHP, nb=NB):
    """Build the 8 per-core input dicts."""
    x_in = np.asarray(inputs["x_in"], np.float32)
    mass = np.asarray(inputs["mass"], np.float32)
    evals = np.asarray(inputs["evals"], np.float32)
    evecs = np.asarray(inputs["evecs"], np.float32)
    rows = np.asarray(inputs["rows"])
    cols = np.asarray(inputs["cols"])
    gX_vals = np.asarray(inputs["gradX_vals"], np.float32)
    gY_vals = np.asarray(inputs["gradY_vals"], np.float32)
    w_first = np.asarray(inputs["w_first"], np.float32)
    b_first = np.asarray(inputs["b_first"], np.float32)
    diff_time = np.asarray(inputs["diff_time"], np.float32)
    A_re = np.asarray(inputs["A_re"], np.float32)
    A_im = np.asarray(inputs["A_im"], np.float32)
    mlp_w0 = np.asarray(inputs["mlp_w0"], np.float32)
    w1 = np.asarray(inputs["mlp_w1"], np.float32)
    w2 = np.asarray(inputs["mlp_w2"], np.float32)
    b0 = np.asarray(inputs["mlp_b0"], np.float32)
    b1 = np.asarray(inputs["mlp_b1"], np.float32)
    b2 = np.asarray(inputs["mlp_b2"], np.float32)
    w_last = np.asarray(inputs["w_last"], np.float32)
    b_last = np.asarray(inputs["b_last"], np.float32)

    nh = NH

    shared = dict(
        Are=A_re[:nb],
        Aim=A_im[:nb],
        w0af=np.ascontiguousarray(mlp_w0[:nb, 0:C]),
        w0bf=np.ascontiguousarray(mlp_w0[:nb, C:2 * C]),
        w0c=mlp_w0[:nb, 2 * C:3 * C].astype(bf16),
        w1=w1[:nb].astype(bf16),
        w2=w2[:nb].astype(bf16),
        b0=b0[:nb].reshape(nb, C, 1),
        b1=b1[:nb].reshape(nb, C, 1),
        b2=b2[:nb].reshape(nb, C, 1),
        wlastf=w_last,
        blast=b_last.reshape(3, 1),
    )

    in_maps = []
    for b in range(B):
        ev = evecs[b]
        evm_full = ev * mass[b][:, None]
        GXe = _spmm_mat(rows, cols, gX_vals[b], ev)
        GYe = _spmm_mat(rows, cols, gY_vals[b], ev)
        x0_full = x_in[b] @ w_first + b_first
        # coefsT[i][c,k] = exp(-evals[k] * diff_time[i][c])
        coefsT = np.exp(-evals[b][None, None, :]
                        * diff_time[:nb, :, None]).astype(np.float32)
        for h in range(2):
            sl = slice(h * nh, (h + 1) * nh)

            def padT(M):  # [nh, K] -> [K, nhp]
                out = np.zeros((M.shape[1], nhp), np.float32)
                out[:, :nh] = M[sl].T
                return out

            evmP = np.zeros((nhp, K), np.float32)
            evmP[:nh] = evm_full[sl]
            evm4 = evmP.reshape(nhp // 512, 4, 128, K).transpose(0, 2, 1, 3) \
                       .reshape(nhp // 512, 128, 512)
            x0T = padT(x0_full)
            in_maps.append(dict(
                evm4=evm4.astype(bf16),
                evT=padT(ev).astype(bf16),
                gxT=padT(GXe).astype(bf16),
                gyT=padT(GYe).astype(bf16),
                x0T=x0T,
                coefsT=coefsT,
                **shared,
            ))
    return in_maps


# --------------------------------------------------------------- device side

def build_nc(nb=NB, nch=NCH, ncores=NCORES, collective=True):
    nhp = nch * CH
    nt = nhp // 128
    nc = bacc.Bacc("TRN2", target_bir_lowering=False, debug=False,
                   enable_asserts=True, num_devices=ncores)

    evm4 = nc.dram_tensor("evm4", [nch, 128, 512], BF, kind="ExternalInput")
    evT = nc.dram_tensor("evT", [K, nhp], BF, kind="ExternalInput")
    gxT = nc.dram_tensor("gxT", [K, nhp], BF, kind="ExternalInput")
    gyT = nc.dram_tensor("gyT", [K, nhp], BF, kind="ExternalInput")
    x0T = nc.dram_tensor("x0T", [C, nhp], F32, kind="ExternalInput")
    coefsT = nc.dram_tensor("coefsT", [nb, C, K], F32, kind="ExternalInput")
    Are = nc.dram_tensor("Are", [nb, C, C], F32, kind="ExternalInput")
    Aim = nc.dram_tensor("Aim", [nb, C, C], F32, kind="ExternalInput")
    w0af = nc.dram_tensor("w0af", [nb, C, C], F32, kind="ExternalInput")
    w0bf = nc.dram_tensor("w0bf", [nb, C, C], F32, kind="ExternalInput")
    w0c = nc.dram_tensor("w0c", [nb, C, C], BF, kind="ExternalInput")
    w1 = nc.dram_tensor("w1", [nb, C, C], BF, kind="ExternalInput")
    w2 = nc.dram_tensor("w2", [nb, C, C], BF, kind="ExternalInput")
    b0 = nc.dram_tensor("b0", [nb, C, 1], F32, kind="ExternalInput")
    b1 = nc.dram_tensor("b1", [nb, C, 1], F32, kind="ExternalInput")
    b2 = nc.dram_tensor("b2", [nb, C, 1], F32, kind="ExternalInput")
    wlastf = nc.dram_tensor("wlastf", [C, 3], F32, kind="ExternalInput")
    blast = nc.dram_tensor("blast", [3, 1], F32, kind="ExternalInput")
    yT = nc.dram_tensor("yT", [3, nhp], F32, kind="ExternalOutput")

    with tile.TileContext(nc) as tc:
        with (
            tc.tile_pool(name="consts", bufs=1) as consts,
            tc.tile_pool(name="xpool", bufs=1) as xpool,
            tc.tile_pool(name="stream", bufs=4) as stream,
            tc.tile_pool(name="csb", bufs=3) as csb,
            tc.tile_pool(name="smalls", bufs=2) as smalls,
            tc.tile_pool(name="mm_ps", bufs=7, space="PSUM") as mm_ps,
            tc.tile_pool(name="small_ps", bufs=1, space="PSUM") as small_ps,
            tc.tile_pool(name="dram", bufs=2, space="DRAM") as dram,
        ):
            ident_bf = consts.tile([128, 128], BF, tag="identb")
            make_identity(nc, ident_bf[:])
            ident_f = consts.tile([128, 128], F32, tag="identf")
            make_identity(nc, ident_f[:])

            def cload(src, shape, dt, tag):
                t = consts.tile(shape, dt, tag=tag)
                nc.sync.dma_start(t[:], src)
                return t

            Are_s = [cload(Are[i], [C, C], F32, f"Are{i}") for i in range(nb)]
            Aim_s = [cload(Aim[i], [C, C], F32, f"Aim{i}") for i in range(nb)]
            coefsT_s = [cload(coefsT[i], [C, K], F32, f"cf{i}") for i in range(nb)]
            w0af_s = [cload(w0af[i], [C, C], F32, f"w0af{i}") for i in range(nb)]
            w0bf_s = [cload(w0bf[i], [C, C], F32, f"w0bf{i}") for i in range(nb)]
            w0c_s = [cload(w0c[i], [C, C], BF, f"w0c{i}") for i in range(nb)]
            w1_s = [cload(w1[i], [C, C], BF, f"w1{i}") for i in range(nb)]
            w2_s = [cload(w2[i], [C, C], BF, f"w2{i}") for i in range(nb)]
            b0_s = [cload(b0[i], [C, 1], F32, f"b0{i}") for i in range(nb)]
            b1_s = [cload(b1[i], [C, 1], F32, f"b1{i}") for i in range(nb)]
            b2_s = [cload(b2[i], [C, 1], F32, f"b2{i}") for i in range(nb)]
            wlastf_s = cload(wlastf[:], [C, 3], F32, "wlast")
            blast_s = cload(blast[:], [3, 1], F32, "blast")

            # fp32r copies of the weights used in fp32r matmuls against x
            w0a_r = []
            for i in range(nb):
                t = consts.tile([C, C], F32R, tag=f"w0ar{i}")
                nc.vector.tensor_copy(t[:], w0af_s[i][:])
                w0a_r.append(t)
            wlast_r = consts.tile([C, 3], F32R, tag="wlastr")
            nc.vector.tensor_copy(wlast_r[:], wlastf_s[:])

            xs = []
            for cI in range(nch):
                sl = bass.ts(cI, CH)
                xtmp = stream.tile([C, CH], F32, tag="x0tmp")
                nc.sync.dma_start(xtmp[:], x0T[:, sl])
                xt = xpool.tile([C, CH], F32R, tag=f"x{cI}")
                nc.vector.tensor_copy(xt[:], xtmp[:])
                xs.append(xt)

            for i in range(nb):
                # ---- forward spectral transform: x_spec^T = sum x^T evm ----
                xspec_ps = small_ps.tile([C, K], F32, tag="sps")
                ebuf = None
                for t in range(nt):
                    cI, f = divmod(t, 4)
                    if f == 0:
                        ebuf = stream.tile([128, 512], BF, tag="evm")
                        nc.sync.dma_start(ebuf[:], evm4[cI])
                    tp = mm_ps.tile([128, 128], F32, tag="mm")
                    nc.tensor.transpose(
                        tp[:], xs[cI][:, f * 128:(f + 1) * 128].bitcast(F32),
                        ident_f[:])
                    xt = csb.tile([128, 128], BF, tag="xt")
                    nc.vector.tensor_copy(xt[:], tp[:])
                    nc.tensor.matmul(xspec_ps[:], xt[:],
                                     ebuf[:, f * 128:(f + 1) * 128],
                                     start=(t == 0), stop=(t == nt - 1))

                # coefs multiply commutes with the pairwise sum -> do it
                # before the AllReduce (off the post-collective critical path)
                STf_p = smalls.tile([C, K], F32, tag="xsp")
                nc.vector.tensor_mul(STf_p[:], xspec_ps[:], coefsT_s[i][:])
                if collective:
                    cc_in = dram.tile([C, K], F32, tag="ccin")
                    cc_out = dram.tile([C, K], F32, tag="ccout")
                    nc.sync.dma_start(cc_in[:], STf_p[:])
                    nc.gpsimd.collective_compute(
                        "AllReduce", ALU.add,
                        replica_groups=PAIRS[:ncores // 2],
                        ins=[cc_in.opt()], outs=[cc_out.opt()])
                    STf = smalls.tile([C, K], F32, tag="STf")
                    nc.sync.dma_start(STf[:], cc_out[:])
                else:
                    STf = STf_p

                # ---- S, its A_re/A_im products, S@w0b ----
                S_ps = small_ps.tile([K, C], F32, tag="sps")
                nc.tensor.transpose(S_ps[:], STf[:], ident_f[:])
                S_bf = smalls.tile([K, C], BF, tag="Sbf")
                nc.scalar.activation(S_bf[:], S_ps[:], AF.Copy)
                Sre_ps = small_ps.tile([K, C], F32, tag="sps")
                nc.tensor.matmul(Sre_ps[:], STf[:], Are_s[i][:],
                                 start=True, stop=True)
                Sre_bf = smalls.tile([K, C], BF, tag="Srebf")
                nc.scalar.activation(Sre_bf[:], Sre_ps[:], AF.Copy)
                Sim_ps = small_ps.tile([K, C], F32, tag="sps")
                nc.tensor.matmul(Sim_ps[:], STf[:], Aim_s[i][:],
                                 start=True, stop=True)
                Sim_bf = smalls.tile([K, C], BF, tag="Simbf")
                nc.scalar.activation(Sim_bf[:], Sim_ps[:], AF.Copy)
                nSim_bf = smalls.tile([K, C], BF, tag="nSimbf")
                nc.vector.tensor_scalar_mul(nSim_bf[:], Sim_ps[:], -1.0)
                SW0b_ps = small_ps.tile([K, C], F32, tag="sps")
                nc.tensor.matmul(SW0b_ps[:], STf[:], w0bf_s[i][:],
                                 start=True, stop=True)
                SW0b_bf = smalls.tile([K, C], BF, tag="SW0b")
                nc.scalar.activation(SW0b_bf[:], SW0b_ps[:], AF.Copy)

                # ---- fused per-node sweep ----
                for cI in range(nch):
                    sl = bass.ts(cI, CH)
                    ev_c = stream.tile([K, CH], BF, tag="ev")
                    nc.sync.dma_start(ev_c[:], evT[:, sl])
                    gx_c = stream.tile([K, CH], BF, tag="gx")
                    nc.sync.dma_start(gx_c[:], gxT[:, sl])
                    gy_c = stream.tile([K, CH], BF, tag="gy")
                    nc.sync.dma_start(gy_c[:], gyT[:, sl])

                    gX_ps = mm_ps.tile([C, CH], F32, tag="mm")
                    nc.tensor.matmul(gX_ps[:], S_bf[:], gx_c[:],
                                     start=True, stop=True)
                    gY_ps = mm_ps.tile([C, CH], F32, tag="mm")
                    nc.tensor.matmul(gY_ps[:], S_bf[:], gy_c[:],
                                     start=True, stop=True)
                    Br_ps = mm_ps.tile([C, CH], F32, tag="mm")
                    nc.tensor.matmul(Br_ps[:], Sre_bf[:], gx_c[:],
                                     start=True, stop=False)
                    nc.tensor.matmul(Br_ps[:], nSim_bf[:], gy_c[:],
                                     start=False, stop=True)
                    Bi_ps = mm_ps.tile([C, CH], F32, tag="mm")
                    nc.tensor.matmul(Bi_ps[:], Sre_bf[:], gy_c[:],
                                     start=True, stop=False)
                    nc.tensor.matmul(Bi_ps[:], Sim_bf[:], gx_c[:],
                                     start=False, stop=True)

                    Br_sb = csb.tile([C, CH], BF, tag="Br")
                    nc.scalar.activation(Br_sb[:], Br_ps[:], AF.Copy)
                    Bi_sb = csb.tile([C, CH], BF, tag="Bi")
                    nc.vector.tensor_copy(Bi_sb[:], Bi_ps[:])
                    m1 = csb.tile([C, CH], BF, tag="m1")
                    nc.vector.tensor_mul(m1[:], gX_ps[:], Br_sb[:])
                    m2 = csb.tile([C, CH], BF, tag="m2")
                    nc.vector.tensor_mul(m2[:], gY_ps[:], Bi_sb[:])
                    a1 = csb.tile([C, CH], BF, tag="a1")
                    nc.vector.tensor_add(a1[:], m1[:], m2[:])
                    gf = csb.tile([C, CH], BF, tag="gf")
                    nc.scalar.activation(gf[:], a1[:], AF.Tanh)

                    h0_ps = mm_ps.tile([C, CH], F32, tag="mm")
                    nc.tensor.matmul(h0_ps[:], w0a_r[i][:], xs[cI][:],
                                     start=True, stop=False)
                    nc.tensor.matmul(h0_ps[:], SW0b_bf[:], ev_c[:],
                                     start=False, stop=False)
                    nc.tensor.matmul(h0_ps[:], w0c_s[i][:], gf[:],
                                     start=False, stop=True)
                    h0_sb = csb.tile([C, CH], BF, tag="h0")
                    nc.scalar.activation(h0_sb[:], h0_ps[:], AF.Relu,
                                         bias=b0_s[i][:])
                    h1_ps = mm_ps.tile([C, CH], F32, tag="mm")
                    nc.tensor.matmul(h1_ps[:], w1_s[i][:], h0_sb[:],
                                     start=True, stop=True)
                    h1_sb = csb.tile([C, CH], BF, tag="h1")
                    nc.scalar.activation(h1_sb[:], h1_ps[:], AF.Relu,
                                         bias=b1_s[i][:])
                    h2_ps = mm_ps.tile([C, CH], F32, tag="mm")
                    nc.tensor.matmul(h2_ps[:], w2_s[i][:], h1_sb[:],
                                     start=True, stop=True)
                    # x += h2 + b2 (x carried in fp32r)
                    nc.vector.scalar_tensor_tensor(
                        out=xs[cI][:], in0=h2_ps[:], scalar=b2_s[i][:],
                        in1=xs[cI][:], op0=ALU.add, op1=ALU.add)

            # ---- output head ----
            for cI in range(nch):
                sl = bass.ts(cI, CH)
                y_ps = mm_ps.tile([3, CH], F32, tag="mm")
                nc.tensor.matmul(y_ps[:], wlast_r[:], xs[cI][:],
                                 start=True, stop=True)
                y_sb = csb.tile([3, CH], F32, tag="y")
                nc.vector.tensor_scalar_add(y_sb[:], y_ps[:], blast_s[:])
                nc.sync.dma_start(yT[:, sl], y_sb[:])

    nc.compile()
    return nc


_NC_CACHE = {}


def _get_nc():
    if "nc" not in _NC_CACHE:
        _NC_CACHE["nc"] = build_nc()
    return _NC_CACHE["nc"]


def kernel(**inputs):
    nc = _get_nc()
    in_maps = host_prep(inputs)
    res = run_bass_kernel_spmd(nc, in_maps, core_ids=list(range(NCORES)))
    out = np.empty((B, N, 3), np.float32)
    for b in range(B):
        for h in range(2):
            yT = res.results[2 * b + h]["yT"]
            out[b, h * NH:(h + 1) * NH] = yT[:, :NH].T
    return out



# revision 2
# speedup vs baseline: 1.0714x; 1.0714x over previous
"""DiffusionNet forward on 8 Trainium2 NeuronCores.

Strategy
--------
B=4 samples, 2 cores per sample, each core owns half the mesh nodes
(20000, zero-padded to 20096 = 157*128).  All cross-node coupling flows
through the K=128 spectral bottleneck:

  * SpMM is eliminated on-device: gX = G @ x_diffuse = (G @ evecs) @ S with
    S = coefs * x_spec, so host precomputes GXe = G @ evecs once per sample
    (exact associativity).
  * Static per-node operands (evm node-major, evecs K-major, GXe K-major)
    are cached in SBUF once; only GYe re-streams per block.  x is carried
    in bf16 channel-major SBUF tiles.
  * The forward spectral transform for block i+1 is piggybacked into the
    per-chunk sweep of block i, so each block boundary is only the 64KB
    pairwise AllReduce plus a short S-matmul chain.
"""

import sys
import numpy as np
import ml_dtypes

for _p in ("/opt/trn_rl_repo", "/root/.axon_site/_ro/trn_rl_repo"):
    if _p not in sys.path:
        sys.path.append(_p)

import concourse.bass as bass
import concourse.bacc as bacc
import concourse.tile as tile
import concourse.mybir as mybir
from concourse.bass_utils import run_bass_kernel_spmd
from concourse.masks import make_identity

BF = mybir.dt.bfloat16
F32 = mybir.dt.float32
AF = mybir.ActivationFunctionType
ALU = mybir.AluOpType

B, N, E, K = 4, 40000, 240000, 128
C = 128
NB = 4          # diffusion blocks
NCORES = 8
NH = N // 2     # nodes per core (half sample)
NHP = 20096     # padded nodes per core: 157 tiles * 128
NT = NHP // 128
# 39 full 512-wide chunks + one 128-wide tail chunk
CHUNKS = [(c * 512, 512) for c in range(39)] + [(39 * 512, 128)]
PAIRS = [[0, 1], [2, 3], [4, 5], [6, 7]]

bf16 = ml_dtypes.bfloat16


# ----------------------------------------------------------------- host side

def _spmm_mat(rows, cols, vals, M):
    """(COO [N,N] with given pattern) @ M, dense M [N,k]. Pure numpy."""
    out = np.zeros((N, M.shape[1]), np.float32)
    perm = np.argsort(rows, kind="stable")
    contrib = (vals[:, None] * M[cols]).astype(np.float32)[perm]
    rs = rows[perm]
    uniq, starts = np.unique(rs, return_index=True)
    out[uniq] = np.add.reduceat(contrib, starts, axis=0)
    return out


def host_prep(inputs, nhp=NHP, nb=NB):
    """Build the 8 per-core input dicts."""
    x_in = np.asarray(inputs["x_in"], np.float32)
    mass = np.asarray(inputs["mass"], np.float32)
    evals = np.asarray(inputs["evals"], np.float32)
    evecs = np.asarray(inputs["evecs"], np.float32)
    rows = np.asarray(inputs["rows"])
    cols = np.asarray(inputs["cols"])
    gX_vals = np.asarray(inputs["gradX_vals"], np.float32)
    gY_vals = np.asarray(inputs["gradY_vals"], np.float32)
    w_first = np.asarray(inputs["w_first"], np.float32)
    b_first = np.asarray(inputs["b_first"], np.float32)
    diff_time = np.asarray(inputs["diff_time"], np.float32)
    A_re = np.asarray(inputs["A_re"], np.float32)
    A_im = np.asarray(inputs["A_im"], np.float32)
    mlp_w0 = np.asarray(inputs["mlp_w0"], np.float32)
    w1 = np.asarray(inputs["mlp_w1"], np.float32)
    w2 = np.asarray(inputs["mlp_w2"], np.float32)
    b0 = np.asarray(inputs["mlp_b0"], np.float32)
    b1 = np.asarray(inputs["mlp_b1"], np.float32)
    b2 = np.asarray(inputs["mlp_b2"], np.float32)
    w_last = np.asarray(inputs["w_last"], np.float32)
    b_last = np.asarray(inputs["b_last"], np.float32)

    nh = NH

    shared = dict(
        Are=A_re[:nb].astype(bf16),
        Aim=A_im[:nb].astype(bf16),
        w0a=np.ascontiguousarray(mlp_w0[:nb, 0:C]).astype(bf16),
        w0b=np.ascontiguousarray(mlp_w0[:nb, C:2 * C]).astype(bf16),
        w0c=np.ascontiguousarray(mlp_w0[:nb, 2 * C:3 * C]).astype(bf16),
        w1=w1[:nb].astype(bf16),
        w2=w2[:nb].astype(bf16),
        b0=b0[:nb].reshape(nb, C, 1),
        b1=b1[:nb].reshape(nb, C, 1),
        b2=b2[:nb].reshape(nb, C, 1),
        wlast=w_last.astype(bf16),
        blast=b_last.reshape(3, 1),
    )

    in_maps = []
    for b in range(B):
        ev = evecs[b]
        evm_full = ev * mass[b][:, None]
        GXe = _spmm_mat(rows, cols, gX_vals[b], ev)
        GYe = _spmm_mat(rows, cols, gY_vals[b], ev)
        x0_full = x_in[b] @ w_first + b_first
        # coefsT[i][c,k] = exp(-evals[k] * diff_time[i][c])
        coefsT = np.exp(-evals[b][None, None, :]
                        * diff_time[:nb, :, None]).astype(np.float32)
        for h in range(2):
            sl = slice(h * nh, (h + 1) * nh)

            def padT(M):  # [nh, K] -> [K, nhp]
                out = np.zeros((M.shape[1], nhp), np.float32)
                out[:, :nh] = M[sl].T
                return out

            evmP = np.zeros((nhp, K), np.float32)
            evmP[:nh] = evm_full[sl]
            # node-major tiles: evm_nm[p, t*128+k] = evmP[t*128+p, k]
            evm_nm = evmP.reshape(NT, 128, K).transpose(1, 0, 2) \
                         .reshape(128, NT * K)
            in_maps.append(dict(
                evmT=evm_nm.astype(bf16),
                evT=padT(ev).astype(bf16),
                gxT=padT(GXe).astype(bf16),
                gyT=padT(GYe).astype(bf16),
                x0T=padT(x0_full).astype(bf16),
                coefsT=coefsT,
                **shared,
            ))
    return in_maps


# --------------------------------------------------------------- device side

def build_nc(nb=NB, ncores=NCORES, collective=True):
    nhp = NHP
    nc = bacc.Bacc("TRN2", target_bir_lowering=False, debug=False,
                   enable_asserts=True, num_devices=ncores)

    evmT = nc.dram_tensor("evmT", [128, nhp], BF, kind="ExternalInput")
    evT = nc.dram_tensor("evT", [K, nhp], BF, kind="ExternalInput")
    gxT = nc.dram_tensor("gxT", [K, nhp], BF, kind="ExternalInput")
    gyT = nc.dram_tensor("gyT", [K, nhp], BF, kind="ExternalInput")
    x0T = nc.dram_tensor("x0T", [C, nhp], BF, kind="ExternalInput")
    coefsT = nc.dram_tensor("coefsT", [nb, C, K], F32, kind="ExternalInput")
    Are = nc.dram_tensor("Are", [nb, C, C], BF, kind="ExternalInput")
    Aim = nc.dram_tensor("Aim", [nb, C, C], BF, kind="ExternalInput")
    w0a = nc.dram_tensor("w0a", [nb, C, C], BF, kind="ExternalInput")
    w0b = nc.dram_tensor("w0b", [nb, C, C], BF, kind="ExternalInput")
    w0c = nc.dram_tensor("w0c", [nb, C, C], BF, kind="ExternalInput")
    w1 = nc.dram_tensor("w1", [nb, C, C], BF, kind="ExternalInput")
    w2 = nc.dram_tensor("w2", [nb, C, C], BF, kind="ExternalInput")
    b0 = nc.dram_tensor("b0", [nb, C, 1], F32, kind="ExternalInput")
    b1 = nc.dram_tensor("b1", [nb, C, 1], F32, kind="ExternalInput")
    b2 = nc.dram_tensor("b2", [nb, C, 1], F32, kind="ExternalInput")
    wlast = nc.dram_tensor("wlast", [C, 3], BF, kind="ExternalInput")
    blast = nc.dram_tensor("blast", [3, 1], F32, kind="ExternalInput")
    yT = nc.dram_tensor("yT", [3, nhp], F32, kind="ExternalOutput")

    with tile.TileContext(nc) as tc:
        with (
            tc.tile_pool(name="consts", bufs=1) as consts,
            tc.tile_pool(name="xpool", bufs=1) as xpool,
            tc.tile_pool(name="gyp", bufs=4) as gyp,
            tc.tile_pool(name="smalls", bufs=2) as smalls,
            tc.tile_pool(name="csb", bufs=2) as csb,
            tc.tile_pool(name="mm_ps", bufs=6, space="PSUM") as mm_ps,
            tc.tile_pool(name="fwd_ps", bufs=1, space="PSUM") as fwd_ps,
            tc.tile_pool(name="spec_ps", bufs=1, space="PSUM") as spec_ps,
            tc.tile_pool(name="dram", bufs=2, space="DRAM") as dram,
        ):
            ident_bf = consts.tile([128, 128], BF, tag="identb")
            make_identity(nc, ident_bf[:])

            def cload(src, shape, dt, tag):
                t = consts.tile(shape, dt, tag=tag)
                nc.sync.dma_start(t[:], src)
                return t

            Are_s = [cload(Are[i], [C, C], BF, f"Are{i}") for i in range(nb)]
            Aim_s = [cload(Aim[i], [C, C], BF, f"Aim{i}") for i in range(nb)]
            coefsT_s = [cload(coefsT[i], [C, K], F32, f"cf{i}") for i in range(nb)]
            w0a_s = [cload(w0a[i], [C, C], BF, f"w0a{i}") for i in range(nb)]
            w0b_s = [cload(w0b[i], [C, C], BF, f"w0b{i}") for i in range(nb)]
            w0c_s = [cload(w0c[i], [C, C], BF, f"w0c{i}") for i in range(nb)]
            w1_s = [cload(w1[i], [C, C], BF, f"w1{i}") for i in range(nb)]
            w2_s = [cload(w2[i], [C, C], BF, f"w2{i}") for i in range(nb)]
            b0_s = [cload(b0[i], [C, 1], F32, f"b0{i}") for i in range(nb)]
            b1_s = [cload(b1[i], [C, 1], F32, f"b1{i}") for i in range(nb)]
            b2_s = [cload(b2[i], [C, 1], F32, f"b2{i}") for i in range(nb)]
            wlast_s = cload(wlast[:], [C, 3], BF, "wlast")
            blast_s = cload(blast[:], [3, 1], F32, "blast")

            # ---- SBUF caches for the static streams (stripe the DMAs so
            # they spread across queues) ----
            NSTRIPE = 8
            stripe = nhp // NSTRIPE  # 2512

            def cache(src, tag):
                t = consts.tile([128, nhp], BF, tag=tag)
                for s in range(NSTRIPE):
                    sl = slice(s * stripe, (s + 1) * stripe)
                    nc.sync.dma_start(t[:, sl], src[:, sl])
                return t

            evm_c = cache(evmT, "evmc")
            ev_c = cache(evT, "evc")
            gx_c = cache(gxT, "gxc")

            # x state, channel-major bf16, one tile per chunk
            xs = []
            for cI, (col0, w) in enumerate(CHUNKS):
                xt = xpool.tile([C, w], BF, tag=f"x{cI}")
                nc.sync.dma_start(xt[:], x0T[:, col0:col0 + w])
                xs.append(xt)

            # ---- piggybacked forward spectral transform for one chunk ----
            def fwd_chunk(cI, col0, w, ispec, parity):
                ntl = w // 128
                ft = fwd_ps.tile([128, w], BF, tag="fwdT", padded_shape=[128, 512])
                for f in range(ntl):
                    nc.tensor.transpose(ft[:, f * 128:(f + 1) * 128],
                                        xs[cI][:, f * 128:(f + 1) * 128],
                                        ident_bf[:])
                xT = csb.tile([128, w], BF, tag="xT", padded_shape=[128, 512])
                if parity:
                    nc.vector.tensor_copy(xT[:], ft[:])
                else:
                    nc.scalar.activation(xT[:], ft[:], AF.Copy)
                t0 = col0 // 128
                for f in range(ntl):
                    t = t0 + f
                    nc.tensor.matmul(ispec[:], xT[:, f * 128:(f + 1) * 128],
                                     evm_c[:, t * 128:(t + 1) * 128],
                                     start=(t == 0), stop=(t == NT - 1))

            xspec = spec_ps.tile([C, K], F32, tag="xspec")
            for cI, (col0, w) in enumerate(CHUNKS):
                fwd_chunk(cI, col0, w, xspec, parity=cI % 2)

            for i in range(nb):
                # ---- coefs multiply (pre-collective), AllReduce over pair ----
                STf_p = smalls.tile([C, K], F32, tag="STfp")
                nc.vector.tensor_mul(STf_p[:], xspec[:], coefsT_s[i][:])
                if collective:
                    cc_in = dram.tile([C, K], F32, tag="ccin")
                    cc_out = dram.tile([C, K], F32, tag="ccout")
                    nc.sync.dma_start(cc_in[:], STf_p[:])
                    nc.gpsimd.collective_compute(
                        "AllReduce", ALU.add,
                        replica_groups=PAIRS[:ncores // 2],
                        ins=[cc_in.opt()], outs=[cc_out.opt()])
                    STf = smalls.tile([C, K], F32, tag="STf")
                    nc.sync.dma_start(STf[:], cc_out[:])
                else:
                    STf = STf_p

                # ---- S (bf16), its A_re/A_im products, S@w0b ----
                STf_bf = smalls.tile([C, K], BF, tag="STfbf")
                nc.scalar.activation(STf_bf[:], STf[:], AF.Copy)
                S_ps = mm_ps.tile([K, C], BF, tag="mm", padded_shape=[K, 512])
                nc.tensor.transpose(S_ps[:], STf_bf[:], ident_bf[:])
                S_bf = smalls.tile([K, C], BF, tag="Sbf")
                nc.vector.tensor_copy(S_bf[:], S_ps[:])
                Sre_ps = mm_ps.tile([K, C], F32, tag="mm", padded_shape=[K, 512])
                nc.tensor.matmul(Sre_ps[:], STf_bf[:], Are_s[i][:],
                                 start=True, stop=True)
                Sre_bf = smalls.tile([K, C], BF, tag="Srebf")
                nc.scalar.activation(Sre_bf[:], Sre_ps[:], AF.Copy)
                Sim_ps = mm_ps.tile([K, C], F32, tag="mm", padded_shape=[K, 512])
                nc.tensor.matmul(Sim_ps[:], STf_bf[:], Aim_s[i][:],
                                 start=True, stop=True)
                Sim_bf = smalls.tile([K, C], BF, tag="Simbf")
                nc.scalar.activation(Sim_bf[:], Sim_ps[:], AF.Copy)
                nSim_bf = smalls.tile([K, C], BF, tag="nSimbf")
                nc.vector.tensor_scalar_mul(nSim_bf[:], Sim_ps[:], -1.0)
                SW0b_ps = mm_ps.tile([K, C], F32, tag="mm", padded_shape=[K, 512])
                nc.tensor.matmul(SW0b_ps[:], STf_bf[:], w0b_s[i][:],
                                 start=True, stop=True)
                SW0b_bf = smalls.tile([K, C], BF, tag="SW0b")
                nc.scalar.activation(SW0b_bf[:], SW0b_ps[:], AF.Copy)

                if i < nb - 1:
                    xspec = spec_ps.tile([C, K], F32, tag="xspec")

                # ---- fused per-node sweep ----
                for cI, (col0, w) in enumerate(CHUNKS):
                    csl = slice(col0, col0 + w)
                    gy_t = gyp.tile([K, w], BF, tag="gy", padded_shape=[K, 512])
                    nc.sync.dma_start(gy_t[:], gyT[:, csl])
                    gx_t = gx_c[:, csl]
                    ev_t = ev_c[:, csl]

                    gX_ps = mm_ps.tile([C, w], F32, tag="mm", padded_shape=[C, 512])
                    nc.tensor.matmul(gX_ps[:], S_bf[:], gx_t,
                                     start=True, stop=True)
                    gY_ps = mm_ps.tile([C, w], F32, tag="mm", padded_shape=[C, 512])
                    nc.tensor.matmul(gY_ps[:], S_bf[:], gy_t[:],
                                     start=True, stop=True)
                    Br_ps = mm_ps.tile([C, w], F32, tag="mm", padded_shape=[C, 512])
                    nc.tensor.matmul(Br_ps[:], Sre_bf[:], gx_t,
                                     start=True, stop=False)
                    nc.tensor.matmul(Br_ps[:], nSim_bf[:], gy_t[:],
                                     start=False, stop=True)
                    Bi_ps = mm_ps.tile([C, w], F32, tag="mm", padded_shape=[C, 512])
                    nc.tensor.matmul(Bi_ps[:], Sre_bf[:], gy_t[:],
                                     start=True, stop=False)
                    nc.tensor.matmul(Bi_ps[:], Sim_bf[:], gx_t,
                                     start=False, stop=True)

                    Br_sb = csb.tile([C, w], BF, tag="Br", padded_shape=[C, 512])
                    nc.vector.tensor_copy(Br_sb[:], Br_ps[:])
                    Bi_sb = csb.tile([C, w], BF, tag="Bi", padded_shape=[C, 512])
                    nc.scalar.activation(Bi_sb[:], Bi_ps[:], AF.Copy)
                    m1 = csb.tile([C, w], BF, tag="m1", padded_shape=[C, 512])
                    nc.vector.tensor_mul(m1[:], gX_ps[:], Br_sb[:])
                    m2 = csb.tile([C, w], BF, tag="m2", padded_shape=[C, 512])
                    nc.vector.tensor_mul(m2[:], gY_ps[:], Bi_sb[:])
                    a1 = csb.tile([C, w], BF, tag="a1", padded_shape=[C, 512])
                    nc.vector.tensor_add(a1[:], m1[:], m2[:])
                    gf = csb.tile([C, w], BF, tag="gf", padded_shape=[C, 512])
                    nc.scalar.activation(gf[:], a1[:], AF.Tanh)

                    h0_ps = mm_ps.tile([C, w], F32, tag="mm", padded_shape=[C, 512])
                    nc.tensor.matmul(h0_ps[:], w0a_s[i][:], xs[cI][:],
                                     start=True, stop=False)
                    nc.tensor.matmul(h0_ps[:], SW0b_bf[:], ev_t,
                                     start=False, stop=False)
                    nc.tensor.matmul(h0_ps[:], w0c_s[i][:], gf[:],
                                     start=False, stop=True)
                    h0_sb = csb.tile([C, w], BF, tag="h0", padded_shape=[C, 512])
                    nc.scalar.activation(h0_sb[:], h0_ps[:], AF.Relu,
                                         bias=b0_s[i][:])
                    h1_ps = mm_ps.tile([C, w], F32, tag="mm", padded_shape=[C, 512])
                    nc.tensor.matmul(h1_ps[:], w1_s[i][:], h0_sb[:],
                                     start=True, stop=True)
                    h1_sb = csb.tile([C, w], BF, tag="h1", padded_shape=[C, 512])
                    nc.scalar.activation(h1_sb[:], h1_ps[:], AF.Relu,
                                         bias=b1_s[i][:])
                    h2_ps = mm_ps.tile([C, w], F32, tag="mm", padded_shape=[C, 512])
                    nc.tensor.matmul(h2_ps[:], w2_s[i][:], h1_sb[:],
                                     start=True, stop=True)
                    # x += h2 + b2 (bf16 carry)
                    nc.vector.scalar_tensor_tensor(
                        out=xs[cI][:], in0=h2_ps[:], scalar=b2_s[i][:],
                        in1=xs[cI][:], op0=ALU.add, op1=ALU.add)

                    if i < nb - 1:
                        fwd_chunk(cI, col0, w, xspec, parity=cI % 2)
                    else:
                        y_ps = mm_ps.tile([3, w], F32, tag="mm",
                                          padded_shape=[3, 512])
                        nc.tensor.matmul(y_ps[:], wlast_s[:], xs[cI][:],
                                         start=True, stop=True)
                        y_sb = csb.tile([3, w], F32, tag="y",
                                        padded_shape=[3, 512])
                        nc.scalar.activation(y_sb[:], y_ps[:], AF.Identity,
                                             bias=blast_s[:])
                        nc.sync.dma_start(yT[:, csl], y_sb[:])

    nc.compile()
    return nc


_NC_CACHE = {}


def _get_nc():
    if "nc" not in _NC_CACHE:
        _NC_CACHE["nc"] = build_nc()
    return _NC_CACHE["nc"]


def kernel(**inputs):
    nc = _get_nc()
    in_maps = host_prep(inputs)
    res = run_bass_kernel_spmd(nc, in_maps, core_ids=list(range(NCORES)))
    out = np.empty((B, N, 3), np.float32)
    for b in range(B):
        for h in range(2):
            yT = res.results[2 * b + h]["yT"]
            out[b, h * NH:(h + 1) * NH] = yT[:, :NH].T
    return out


# revision 3
# speedup vs baseline: 1.0793x; 1.0073x over previous
"""DiffusionNet forward on 8 Trainium2 NeuronCores.

Strategy
--------
B=4 samples, 2 cores per sample, each core owns half the mesh nodes
(20000, zero-padded to 20096 = 157*128).  All cross-node coupling flows
through the K=128 spectral bottleneck:

  * SpMM is eliminated on-device: gX = G @ x_diffuse = (G @ evecs) @ S with
    S = coefs * x_spec, so host precomputes GXe = G @ evecs once per sample
    (exact associativity).
  * Static per-node operands (evm node-major, evecs K-major, GXe K-major)
    are cached in SBUF once; only GYe re-streams per block.  x is carried
    in bf16 channel-major SBUF tiles.
  * The forward spectral transform for block i+1 is piggybacked into the
    per-chunk sweep of block i (x^T tiles via DMA xbar transposes), so each
    block boundary is only the 64KB pairwise AllReduce plus a short
    S-matmul chain.
  * PSUM banks are assigned per-role (gX/gY/Br/Bi/h0/h1/h2/xspec) and
    SBUF working tiles triple-buffered so ~3 chunks pipeline across
    engines; elementwise work is split between DVE and Act.
"""

import sys
import numpy as np
import ml_dtypes

for _p in ("/opt/trn_rl_repo", "/root/.axon_site/_ro/trn_rl_repo"):
    if _p not in sys.path:
        sys.path.append(_p)

import concourse.bass as bass
import concourse.bacc as bacc
import concourse.tile as tile
import concourse.mybir as mybir
from concourse.bass_utils import run_bass_kernel_spmd
from concourse.masks import make_identity

BF = mybir.dt.bfloat16
F32 = mybir.dt.float32
AF = mybir.ActivationFunctionType
ALU = mybir.AluOpType

B, N, E, K = 4, 40000, 240000, 128
C = 128
NB = 4          # diffusion blocks
NCORES = 8
NH = N // 2     # nodes per core (half sample)
NHP = 20096     # padded nodes per core: 157 tiles * 128
NT = NHP // 128
# 39 full 512-wide chunks + one 128-wide tail chunk
CHUNKS = [(c * 512, 512) for c in range(39)] + [(39 * 512, 128)]
PAIRS = [[0, 1], [2, 3], [4, 5], [6, 7]]
USE_DMA_T = True   # xbar DMA transposes for the forward piggyback

bf16 = ml_dtypes.bfloat16


# ----------------------------------------------------------------- host side

def _spmm_mat(rows, cols, vals, M):
    """(COO [N,N] with given pattern) @ M, dense M [N,k]. Pure numpy."""
    out = np.zeros((N, M.shape[1]), np.float32)
    perm = np.argsort(rows, kind="stable")
    contrib = (vals[:, None] * M[cols]).astype(np.float32)[perm]
    rs = rows[perm]
    uniq, starts = np.unique(rs, return_index=True)
    out[uniq] = np.add.reduceat(contrib, starts, axis=0)
    return out


def host_prep(inputs, nhp=NHP, nb=NB):
    """Build the 8 per-core input dicts."""
    x_in = np.asarray(inputs["x_in"], np.float32)
    mass = np.asarray(inputs["mass"], np.float32)
    evals = np.asarray(inputs["evals"], np.float32)
    evecs = np.asarray(inputs["evecs"], np.float32)
    rows = np.asarray(inputs["rows"])
    cols = np.asarray(inputs["cols"])
    gX_vals = np.asarray(inputs["gradX_vals"], np.float32)
    gY_vals = np.asarray(inputs["gradY_vals"], np.float32)
    w_first = np.asarray(inputs["w_first"], np.float32)
    b_first = np.asarray(inputs["b_first"], np.float32)
    diff_time = np.asarray(inputs["diff_time"], np.float32)
    A_re = np.asarray(inputs["A_re"], np.float32)
    A_im = np.asarray(inputs["A_im"], np.float32)
    mlp_w0 = np.asarray(inputs["mlp_w0"], np.float32)
    w1 = np.asarray(inputs["mlp_w1"], np.float32)
    w2 = np.asarray(inputs["mlp_w2"], np.float32)
    b0 = np.asarray(inputs["mlp_b0"], np.float32)
    b1 = np.asarray(inputs["mlp_b1"], np.float32)
    b2 = np.asarray(inputs["mlp_b2"], np.float32)
    w_last = np.asarray(inputs["w_last"], np.float32)
    b_last = np.asarray(inputs["b_last"], np.float32)

    nh = NH

    shared = dict(
        Are=A_re[:nb].astype(bf16),
        Aim=A_im[:nb].astype(bf16),
        w0a=np.ascontiguousarray(mlp_w0[:nb, 0:C]).astype(bf16),
        w0b=np.ascontiguousarray(mlp_w0[:nb, C:2 * C]).astype(bf16),
        w0c=np.ascontiguousarray(mlp_w0[:nb, 2 * C:3 * C]).astype(bf16),
        w1=w1[:nb].astype(bf16),
        w2=w2[:nb].astype(bf16),
        b0=b0[:nb].reshape(nb, C, 1),
        b1=b1[:nb].reshape(nb, C, 1),
        b2=b2[:nb].reshape(nb, C, 1),
        wlast=w_last.astype(bf16),
        blast=b_last.reshape(3, 1),
    )

    in_maps = []
    for b in range(B):
        ev = evecs[b]
        evm_full = ev * mass[b][:, None]
        GXe = _spmm_mat(rows, cols, gX_vals[b], ev)
        GYe = _spmm_mat(rows, cols, gY_vals[b], ev)
        x0_full = x_in[b] @ w_first + b_first
        # coefsT[i][c,k] = exp(-evals[k] * diff_time[i][c])
        coefsT = np.exp(-evals[b][None, None, :]
                        * diff_time[:nb, :, None]).astype(np.float32)
        for h in range(2):
            sl = slice(h * nh, (h + 1) * nh)

            def padT(M):  # [nh, K] -> [K, nhp]
                out = np.zeros((M.shape[1], nhp), np.float32)
                out[:, :nh] = M[sl].T
                return out

            evmP = np.zeros((nhp, K), np.float32)
            evmP[:nh] = evm_full[sl]
            # node-major tiles: evm_nm[p, t*128+k] = evmP[t*128+p, k]
            evm_nm = evmP.reshape(NT, 128, K).transpose(1, 0, 2) \
                         .reshape(128, NT * K)
            in_maps.append(dict(
                evmT=evm_nm.astype(bf16),
                evT=padT(ev).astype(bf16),
                gxT=padT(GXe).astype(bf16),
                gyT=padT(GYe).astype(bf16),
                x0T=padT(x0_full).astype(bf16),
                coefsT=coefsT,
                **shared,
            ))
    return in_maps


# --------------------------------------------------------------- device side

def build_nc(nb=NB, ncores=NCORES, collective=True):
    nhp = NHP
    nc = bacc.Bacc("TRN2", target_bir_lowering=False, debug=False,
                   enable_asserts=True, num_devices=ncores)

    evmT = nc.dram_tensor("evmT", [128, nhp], BF, kind="ExternalInput")
    evT = nc.dram_tensor("evT", [K, nhp], BF, kind="ExternalInput")
    gxT = nc.dram_tensor("gxT", [K, nhp], BF, kind="ExternalInput")
    gyT = nc.dram_tensor("gyT", [K, nhp], BF, kind="ExternalInput")
    x0T = nc.dram_tensor("x0T", [C, nhp], BF, kind="ExternalInput")
    coefsT = nc.dram_tensor("coefsT", [nb, C, K], F32, kind="ExternalInput")
    Are = nc.dram_tensor("Are", [nb, C, C], BF, kind="ExternalInput")
    Aim = nc.dram_tensor("Aim", [nb, C, C], BF, kind="ExternalInput")
    w0a = nc.dram_tensor("w0a", [nb, C, C], BF, kind="ExternalInput")
    w0b = nc.dram_tensor("w0b", [nb, C, C], BF, kind="ExternalInput")
    w0c = nc.dram_tensor("w0c", [nb, C, C], BF, kind="ExternalInput")
    w1 = nc.dram_tensor("w1", [nb, C, C], BF, kind="ExternalInput")
    w2 = nc.dram_tensor("w2", [nb, C, C], BF, kind="ExternalInput")
    b0 = nc.dram_tensor("b0", [nb, C, 1], F32, kind="ExternalInput")
    b1 = nc.dram_tensor("b1", [nb, C, 1], F32, kind="ExternalInput")
    b2 = nc.dram_tensor("b2", [nb, C, 1], F32, kind="ExternalInput")
    wlast = nc.dram_tensor("wlast", [C, 3], BF, kind="ExternalInput")
    blast = nc.dram_tensor("blast", [3, 1], F32, kind="ExternalInput")
    yT = nc.dram_tensor("yT", [3, nhp], F32, kind="ExternalOutput")

    with tile.TileContext(nc) as tc:
        with (
            tc.tile_pool(name="consts", bufs=1) as consts,
            tc.tile_pool(name="xpool", bufs=1) as xpool,
            tc.tile_pool(name="gyp", bufs=4) as gyp,
            tc.tile_pool(name="smalls", bufs=1) as smalls,
            tc.tile_pool(name="csb", bufs=3) as csb,
            tc.tile_pool(name="ps", bufs=1, space="PSUM") as ps,
            tc.tile_pool(name="dram", bufs=2, space="DRAM") as dram,
        ):
            ident_bf = consts.tile([128, 128], BF, tag="identb")
            make_identity(nc, ident_bf[:])

            def cload(src, shape, dt, tag):
                t = consts.tile(shape, dt, tag=tag)
                nc.sync.dma_start(t[:], src)
                return t

            Are_s = [cload(Are[i], [C, C], BF, f"Are{i}") for i in range(nb)]
            Aim_s = [cload(Aim[i], [C, C], BF, f"Aim{i}") for i in range(nb)]
            coefsT_s = [cload(coefsT[i], [C, K], F32, f"cf{i}") for i in range(nb)]
            w0a_s = [cload(w0a[i], [C, C], BF, f"w0a{i}") for i in range(nb)]
            w0b_s = [cload(w0b[i], [C, C], BF, f"w0b{i}") for i in range(nb)]
            w0c_s = [cload(w0c[i], [C, C], BF, f"w0c{i}") for i in range(nb)]
            w1_s = [cload(w1[i], [C, C], BF, f"w1{i}") for i in range(nb)]
            w2_s = [cload(w2[i], [C, C], BF, f"w2{i}") for i in range(nb)]
            b0_s = [cload(b0[i], [C, 1], F32, f"b0{i}") for i in range(nb)]
            b1_s = [cload(b1[i], [C, 1], F32, f"b1{i}") for i in range(nb)]
            b2_s = [cload(b2[i], [C, 1], F32, f"b2{i}") for i in range(nb)]
            wlast_s = cload(wlast[:], [C, 3], BF, "wlast")
            blast_s = cload(blast[:], [3, 1], F32, "blast")

            # ---- SBUF caches for the static streams (stripe the DMAs so
            # they spread across queues) ----
            NSTRIPE = 8
            stripe = nhp // NSTRIPE  # 2512

            def cache(src, tag):
                t = consts.tile([128, nhp], BF, tag=tag)
                for s in range(NSTRIPE):
                    sl = slice(s * stripe, (s + 1) * stripe)
                    nc.sync.dma_start(t[:, sl], src[:, sl])
                return t

            evm_c = cache(evmT, "evmc")
            ev_c = cache(evT, "evc")
            gx_c = cache(gxT, "gxc")

            # x state, channel-major bf16, one tile per chunk
            xs = []
            for cI, (col0, w) in enumerate(CHUNKS):
                xt = xpool.tile([C, w], BF, tag=f"x{cI}")
                nc.sync.dma_start(xt[:], x0T[:, col0:col0 + w])
                xs.append(xt)

            # ---- forward spectral transform for one chunk (piggybacked) ----
            def fwd_chunk(cI, col0, w, ispec, parity):
                ntl = w // 128
                if USE_DMA_T:
                    xT = csb.tile([128, w], BF, tag="xT",
                                  padded_shape=[128, 512])
                    for f in range(ntl):
                        nc.sync.dma_start_transpose(
                            xT[:, f * 128:(f + 1) * 128],
                            xs[cI][:, f * 128:(f + 1) * 128])
                else:
                    ft = ps.tile([128, w], BF, tag="h1",
                                 padded_shape=[128, 512])
                    for f in range(ntl):
                        nc.tensor.transpose(ft[:, f * 128:(f + 1) * 128],
                                            xs[cI][:, f * 128:(f + 1) * 128],
                                            ident_bf[:])
                    xT = csb.tile([128, w], BF, tag="xT",
                                  padded_shape=[128, 512])
                    if parity:
                        nc.vector.tensor_copy(xT[:], ft[:])
                    else:
                        nc.scalar.activation(xT[:], ft[:], AF.Copy)
                t0 = col0 // 128
                for f in range(ntl):
                    t = t0 + f
                    nc.tensor.matmul(ispec[:], xT[:, f * 128:(f + 1) * 128],
                                     evm_c[:, t * 128:(t + 1) * 128],
                                     start=(t == 0), stop=(t == NT - 1))

            xspec = ps.tile([C, K], F32, tag="xspec")
            for cI, (col0, w) in enumerate(CHUNKS):
                fwd_chunk(cI, col0, w, xspec, parity=cI % 2)

            for i in range(nb):
                # ---- coefs multiply (pre-collective), AllReduce over pair ----
                STf_p = smalls.tile([C, K], F32, tag="STfp")
                nc.vector.tensor_mul(STf_p[:], xspec[:], coefsT_s[i][:])
                if collective:
                    cc_in = dram.tile([C, K], F32, tag="ccin")
                    cc_out = dram.tile([C, K], F32, tag="ccout")
                    nc.sync.dma_start(cc_in[:], STf_p[:])
                    nc.gpsimd.collective_compute(
                        "AllReduce", ALU.add,
                        replica_groups=PAIRS[:ncores // 2],
                        ins=[cc_in.opt()], outs=[cc_out.opt()])
                    STf = smalls.tile([C, K], F32, tag="STf")
                    nc.sync.dma_start(STf[:], cc_out[:])
                else:
                    STf = STf_p

                # ---- S (bf16), its A_re/A_im products, S@w0b ----
                STf_bf = smalls.tile([C, K], BF, tag="STfbf")
                nc.scalar.activation(STf_bf[:], STf[:], AF.Copy)
                S_ps = ps.tile([K, C], BF, tag="gX", padded_shape=[K, 512])
                nc.tensor.transpose(S_ps[:], STf_bf[:], ident_bf[:])
                S_bf = smalls.tile([K, C], BF, tag="Sbf")
                nc.vector.tensor_copy(S_bf[:], S_ps[:])
                Sre_ps = ps.tile([K, C], F32, tag="gY", padded_shape=[K, 512])
                nc.tensor.matmul(Sre_ps[:], STf_bf[:], Are_s[i][:],
                                 start=True, stop=True)
                Sre_bf = smalls.tile([K, C], BF, tag="Srebf")
                nc.scalar.activation(Sre_bf[:], Sre_ps[:], AF.Copy)
                Sim_ps = ps.tile([K, C], F32, tag="Br", padded_shape=[K, 512])
                nc.tensor.matmul(Sim_ps[:], STf_bf[:], Aim_s[i][:],
                                 start=True, stop=True)
                Sim_bf = smalls.tile([K, C], BF, tag="Simbf")
                nc.scalar.activation(Sim_bf[:], Sim_ps[:], AF.Copy)
                nSim_bf = smalls.tile([K, C], BF, tag="nSimbf")
                nc.vector.tensor_scalar_mul(nSim_bf[:], Sim_ps[:], -1.0)
                SW0b_ps = ps.tile([K, C], F32, tag="Bi", padded_shape=[K, 512])
                nc.tensor.matmul(SW0b_ps[:], STf_bf[:], w0b_s[i][:],
                                 start=True, stop=True)
                SW0b_bf = smalls.tile([K, C], BF, tag="SW0b")
                nc.scalar.activation(SW0b_bf[:], SW0b_ps[:], AF.Copy)

                if i < nb - 1:
                    xspec = ps.tile([C, K], F32, tag="xspec")

                # ---- fused per-node sweep ----
                for cI, (col0, w) in enumerate(CHUNKS):
                    csl = slice(col0, col0 + w)
                    gy_t = gyp.tile([K, w], BF, tag="gy", padded_shape=[K, 512])
                    nc.sync.dma_start(gy_t[:], gyT[:, csl])
                    gx_t = gx_c[:, csl]
                    ev_t = ev_c[:, csl]

                    # grad path: Br/Bi first (their copies gate m1/m2)
                    Br_ps = ps.tile([C, w], F32, tag="Br", padded_shape=[C, 512])
                    nc.tensor.matmul(Br_ps[:], Sre_bf[:], gx_t,
                                     start=True, stop=False)
                    nc.tensor.matmul(Br_ps[:], nSim_bf[:], gy_t[:],
                                     start=False, stop=True)
                    Bi_ps = ps.tile([C, w], F32, tag="Bi", padded_shape=[C, 512])
                    nc.tensor.matmul(Bi_ps[:], Sre_bf[:], gy_t[:],
                                     start=True, stop=False)
                    nc.tensor.matmul(Bi_ps[:], Sim_bf[:], gx_t,
                                     start=False, stop=True)
                    Br_sb = csb.tile([C, w], BF, tag="Br", padded_shape=[C, 512])
                    nc.scalar.activation(Br_sb[:], Br_ps[:], AF.Copy)
                    Bi_sb = csb.tile([C, w], BF, tag="Bi", padded_shape=[C, 512])
                    nc.scalar.activation(Bi_sb[:], Bi_ps[:], AF.Copy)

                    gX_ps = ps.tile([C, w], F32, tag="gX", padded_shape=[C, 512])
                    nc.tensor.matmul(gX_ps[:], S_bf[:], gx_t,
                                     start=True, stop=True)
                    m1 = csb.tile([C, w], BF, tag="m1", padded_shape=[C, 512])
                    nc.vector.tensor_mul(m1[:], gX_ps[:], Br_sb[:])
                    gY_ps = ps.tile([C, w], F32, tag="gY", padded_shape=[C, 512])
                    nc.tensor.matmul(gY_ps[:], S_bf[:], gy_t[:],
                                     start=True, stop=True)
                    m2 = csb.tile([C, w], BF, tag="m2", padded_shape=[C, 512])
                    nc.vector.tensor_mul(m2[:], gY_ps[:], Bi_sb[:])
                    a1 = csb.tile([C, w], BF, tag="a1", padded_shape=[C, 512])
                    nc.vector.tensor_add(a1[:], m1[:], m2[:])
                    gf = csb.tile([C, w], BF, tag="gf", padded_shape=[C, 512])
                    nc.scalar.activation(gf[:], a1[:], AF.Tanh)

                    # MLP path: x/ev partial sums early, gf term last
                    h0_ps = ps.tile([C, w], F32, tag="h0", padded_shape=[C, 512])
                    nc.tensor.matmul(h0_ps[:], w0a_s[i][:], xs[cI][:],
                                     start=True, stop=False)
                    nc.tensor.matmul(h0_ps[:], SW0b_bf[:], ev_t,
                                     start=False, stop=False)
                    nc.tensor.matmul(h0_ps[:], w0c_s[i][:], gf[:],
                                     start=False, stop=True)
                    h0_sb = csb.tile([C, w], BF, tag="h0", padded_shape=[C, 512])
                    nc.vector.tensor_scalar(h0_sb[:], h0_ps[:], b0_s[i][:],
                                            0.0, ALU.add, ALU.max)
                    h1_ps = ps.tile([C, w], F32, tag="h1", padded_shape=[C, 512])
                    nc.tensor.matmul(h1_ps[:], w1_s[i][:], h0_sb[:],
                                     start=True, stop=True)
                    h1_sb = csb.tile([C, w], BF, tag="h1", padded_shape=[C, 512])
                    nc.scalar.activation(h1_sb[:], h1_ps[:], AF.Relu,
                                         bias=b1_s[i][:])
                    h2_ps = ps.tile([C, w], F32, tag="h2", padded_shape=[C, 512])
                    nc.tensor.matmul(h2_ps[:], w2_s[i][:], h1_sb[:],
                                     start=True, stop=True)
                    # x += h2 + b2 (bf16 carry)
                    nc.vector.scalar_tensor_tensor(
                        out=xs[cI][:], in0=h2_ps[:], scalar=b2_s[i][:],
                        in1=xs[cI][:], op0=ALU.add, op1=ALU.add)

                    if i < nb - 1:
                        fwd_chunk(cI, col0, w, xspec, parity=cI % 2)
                    else:
                        y_ps = ps.tile([3, w], F32, tag="h0",
                                       padded_shape=[3, 512])
                        nc.tensor.matmul(y_ps[:], wlast_s[:], xs[cI][:],
                                         start=True, stop=True)
                        y_sb = csb.tile([3, w], F32, tag="y",
                                        padded_shape=[3, 512], bufs=2)
                        nc.scalar.activation(y_sb[:], y_ps[:], AF.Identity,
                                             bias=blast_s[:])
                        nc.sync.dma_start(yT[:, csl], y_sb[:])

    nc.compile()
    return nc


_NC_CACHE = {}


def _get_nc():
    if "nc" not in _NC_CACHE:
        _NC_CACHE["nc"] = build_nc()
    return _NC_CACHE["nc"]


def kernel(**inputs):
    nc = _get_nc()
    in_maps = host_prep(inputs)
    res = run_bass_kernel_spmd(nc, in_maps, core_ids=list(range(NCORES)))
    out = np.empty((B, N, 3), np.float32)
    for b in range(B):
        for h in range(2):
            yT = res.results[2 * b + h]["yT"]
            out[b, h * NH:(h + 1) * NH] = yT[:, :NH].T
    return out


# revision 5
# speedup vs baseline: 1.7890x; 1.6577x over previous
"""DiffusionNet forward on 8 Trainium2 NeuronCores.

Strategy
--------
B=4 samples, 2 cores per sample, each core owns half the mesh nodes
(20000, zero-padded to 20096 = 157*128).  All cross-node coupling flows
through the K=128 spectral bottleneck:

  * SpMM is eliminated on-device: gX = G @ x_diffuse = (G @ evecs) @ S with
    S = coefs * x_spec, so host precomputes GXe = G @ evecs once per sample
    (exact associativity).
  * Static per-node operands (evm node-major, evecs K-major, GXe K-major)
    are cached in SBUF once; only GYe re-streams per block.  x is carried
    in bf16 channel-major SBUF tiles.
  * The forward spectral transform for block i+1 is piggybacked into the
    per-chunk sweep of block i (x^T tiles via DMA xbar transposes), so each
    block boundary is only the 64KB pairwise AllReduce plus a short
    S-matmul chain.
  * PSUM banks are assigned per-role (gX/gY/Br/Bi/h0/h1/h2/xspec) and
    SBUF working tiles triple-buffered so ~3 chunks pipeline across
    engines; elementwise work is split between DVE and Act.
"""

import sys
import numpy as np
import ml_dtypes

for _p in ("/opt/trn_rl_repo", "/root/.axon_site/_ro/trn_rl_repo"):
    if _p not in sys.path:
        sys.path.append(_p)

import concourse.bass as bass
import concourse.bacc as bacc
import concourse.tile as tile
import concourse.mybir as mybir
from concourse.bass_utils import run_bass_kernel_spmd
from concourse.masks import make_identity

BF = mybir.dt.bfloat16
F32 = mybir.dt.float32
AF = mybir.ActivationFunctionType
ALU = mybir.AluOpType

B, N, E, K = 4, 40000, 240000, 128
C = 128
NB = 4          # diffusion blocks
NCORES = 8
NH = N // 2     # nodes per core (half sample)
NHP = 20096     # padded nodes per core: 157 tiles * 128
NT = NHP // 128
# 39 full 512-wide chunks + one 128-wide tail chunk
CHUNKS = [(c * 512, 512) for c in range(39)] + [(39 * 512, 128)]
PAIRS = [[0, 1], [2, 3], [4, 5], [6, 7]]
USE_DMA_T = True   # xbar DMA transposes for the forward piggyback

bf16 = ml_dtypes.bfloat16


# ----------------------------------------------------------------- host side

def _spmm_mat(rows, cols, vals, M):
    """(COO [N,N] with given pattern) @ M, dense M [N,k]. Pure numpy."""
    out = np.zeros((N, M.shape[1]), np.float32)
    perm = np.argsort(rows, kind="stable")
    contrib = (vals[:, None] * M[cols]).astype(np.float32)[perm]
    rs = rows[perm]
    uniq, starts = np.unique(rs, return_index=True)
    out[uniq] = np.add.reduceat(contrib, starts, axis=0)
    return out


def host_prep(inputs, nhp=NHP, nb=NB):
    """Build the 8 per-core input dicts."""
    x_in = np.asarray(inputs["x_in"], np.float32)
    mass = np.asarray(inputs["mass"], np.float32)
    evals = np.asarray(inputs["evals"], np.float32)
    evecs = np.asarray(inputs["evecs"], np.float32)
    rows = np.asarray(inputs["rows"])
    cols = np.asarray(inputs["cols"])
    gX_vals = np.asarray(inputs["gradX_vals"], np.float32)
    gY_vals = np.asarray(inputs["gradY_vals"], np.float32)
    w_first = np.asarray(inputs["w_first"], np.float32)
    b_first = np.asarray(inputs["b_first"], np.float32)
    diff_time = np.asarray(inputs["diff_time"], np.float32)
    A_re = np.asarray(inputs["A_re"], np.float32)
    A_im = np.asarray(inputs["A_im"], np.float32)
    mlp_w0 = np.asarray(inputs["mlp_w0"], np.float32)
    w1 = np.asarray(inputs["mlp_w1"], np.float32)
    w2 = np.asarray(inputs["mlp_w2"], np.float32)
    b0 = np.asarray(inputs["mlp_b0"], np.float32)
    b1 = np.asarray(inputs["mlp_b1"], np.float32)
    b2 = np.asarray(inputs["mlp_b2"], np.float32)
    w_last = np.asarray(inputs["w_last"], np.float32)
    b_last = np.asarray(inputs["b_last"], np.float32)

    nh = NH

    shared = dict(
        Are=A_re[:nb].astype(bf16),
        Aim=A_im[:nb].astype(bf16),
        w0a=np.ascontiguousarray(mlp_w0[:nb, 0:C]).astype(bf16),
        w0b=np.ascontiguousarray(mlp_w0[:nb, C:2 * C]).astype(bf16),
        w0c=np.ascontiguousarray(mlp_w0[:nb, 2 * C:3 * C]).astype(bf16),
        w1=w1[:nb].astype(bf16),
        w2=w2[:nb].astype(bf16),
        b0=b0[:nb].reshape(nb, C, 1),
        b1=b1[:nb].reshape(nb, C, 1),
        b2=b2[:nb].reshape(nb, C, 1),
        wlast=w_last.astype(bf16),
        blast=b_last.reshape(3, 1),
    )

    in_maps = []
    for b in range(B):
        ev = evecs[b]
        evm_full = ev * mass[b][:, None]
        GXe = _spmm_mat(rows, cols, gX_vals[b], ev)
        GYe = _spmm_mat(rows, cols, gY_vals[b], ev)
        x0_full = x_in[b] @ w_first + b_first
        # coefsT[i][c,k] = exp(-evals[k] * diff_time[i][c])
        coefsT = np.exp(-evals[b][None, None, :]
                        * diff_time[:nb, :, None]).astype(np.float32)
        for h in range(2):
            sl = slice(h * nh, (h + 1) * nh)

            def padT(M):  # [nh, K] -> [K, nhp]
                out = np.zeros((M.shape[1], nhp), np.float32)
                out[:, :nh] = M[sl].T
                return out

            evmP = np.zeros((nhp, K), np.float32)
            evmP[:nh] = evm_full[sl]
            # node-major tiles: evm_nm[p, t*128+k] = evmP[t*128+p, k]
            evm_nm = evmP.reshape(NT, 128, K).transpose(1, 0, 2) \
                         .reshape(128, NT * K)
            in_maps.append(dict(
                evmT=evm_nm.astype(bf16),
                evT=padT(ev).astype(bf16),
                gxT=padT(GXe).astype(bf16),
                gyT=padT(GYe).astype(bf16),
                x0T=padT(x0_full).astype(bf16),
                coefsT=coefsT,
                **shared,
            ))
    return in_maps


# --------------------------------------------------------------- device side

def build_nc(nb=NB, ncores=NCORES, collective=True):
    nhp = NHP
    nc = bacc.Bacc("TRN2", target_bir_lowering=False, debug=False,
                   enable_asserts=True, num_devices=ncores)

    evmT = nc.dram_tensor("evmT", [128, nhp], BF, kind="ExternalInput")
    evT = nc.dram_tensor("evT", [K, nhp], BF, kind="ExternalInput")
    gxT = nc.dram_tensor("gxT", [K, nhp], BF, kind="ExternalInput")
    gyT = nc.dram_tensor("gyT", [K, nhp], BF, kind="ExternalInput")
    x0T = nc.dram_tensor("x0T", [C, nhp], BF, kind="ExternalInput")
    coefsT = nc.dram_tensor("coefsT", [nb, C, K], F32, kind="ExternalInput")
    Are = nc.dram_tensor("Are", [nb, C, C], BF, kind="ExternalInput")
    Aim = nc.dram_tensor("Aim", [nb, C, C], BF, kind="ExternalInput")
    w0a = nc.dram_tensor("w0a", [nb, C, C], BF, kind="ExternalInput")
    w0b = nc.dram_tensor("w0b", [nb, C, C], BF, kind="ExternalInput")
    w0c = nc.dram_tensor("w0c", [nb, C, C], BF, kind="ExternalInput")
    w1 = nc.dram_tensor("w1", [nb, C, C], BF, kind="ExternalInput")
    w2 = nc.dram_tensor("w2", [nb, C, C], BF, kind="ExternalInput")
    b0 = nc.dram_tensor("b0", [nb, C, 1], F32, kind="ExternalInput")
    b1 = nc.dram_tensor("b1", [nb, C, 1], F32, kind="ExternalInput")
    b2 = nc.dram_tensor("b2", [nb, C, 1], F32, kind="ExternalInput")
    wlast = nc.dram_tensor("wlast", [C, 3], BF, kind="ExternalInput")
    blast = nc.dram_tensor("blast", [3, 1], F32, kind="ExternalInput")
    yT = nc.dram_tensor("yT", [3, nhp], F32, kind="ExternalOutput")

    with tile.TileContext(nc) as tc:
        with (
            tc.tile_pool(name="consts", bufs=1) as consts,
            tc.tile_pool(name="xpool", bufs=1) as xpool,
            tc.tile_pool(name="gyp", bufs=4) as gyp,
            tc.tile_pool(name="smalls", bufs=1) as smalls,
            tc.tile_pool(name="csb", bufs=3) as csb,
            tc.tile_pool(name="ps", bufs=1, space="PSUM") as ps,
            tc.tile_pool(name="dram", bufs=2, space="DRAM") as dram,
        ):
            ident_bf = consts.tile([128, 128], BF, tag="identb")
            make_identity(nc, ident_bf[:])

            def cload(src, shape, dt, tag):
                t = consts.tile(shape, dt, tag=tag)
                nc.sync.dma_start(t[:], src)
                return t

            Are_s = [cload(Are[i], [C, C], BF, f"Are{i}") for i in range(nb)]
            Aim_s = [cload(Aim[i], [C, C], BF, f"Aim{i}") for i in range(nb)]
            coefsT_s = [cload(coefsT[i], [C, K], F32, f"cf{i}") for i in range(nb)]
            w0a_s = [cload(w0a[i], [C, C], BF, f"w0a{i}") for i in range(nb)]
            w0b_s = [cload(w0b[i], [C, C], BF, f"w0b{i}") for i in range(nb)]
            w0c_s = [cload(w0c[i], [C, C], BF, f"w0c{i}") for i in range(nb)]
            w1_s = [cload(w1[i], [C, C], BF, f"w1{i}") for i in range(nb)]
            w2_s = [cload(w2[i], [C, C], BF, f"w2{i}") for i in range(nb)]
            b0_s = [cload(b0[i], [C, 1], F32, f"b0{i}") for i in range(nb)]
            b1_s = [cload(b1[i], [C, 1], F32, f"b1{i}") for i in range(nb)]
            b2_s = [cload(b2[i], [C, 1], F32, f"b2{i}") for i in range(nb)]
            wlast_s = cload(wlast[:], [C, 3], BF, "wlast")
            blast_s = cload(blast[:], [3, 1], F32, "blast")

            # ---- SBUF caches for the static streams (stripe the DMAs so
            # they spread across queues) ----
            NSTRIPE = 8
            stripe = nhp // NSTRIPE  # 2512

            def cache(src, tag):
                t = consts.tile([128, nhp], BF, tag=tag)
                for s in range(NSTRIPE):
                    sl = slice(s * stripe, (s + 1) * stripe)
                    nc.sync.dma_start(t[:, sl], src[:, sl])
                return t

            evm_c = cache(evmT, "evmc")
            ev_c = cache(evT, "evc")
            gx_c = cache(gxT, "gxc")

            # x state, channel-major bf16, one tile per chunk
            xs = []
            for cI, (col0, w) in enumerate(CHUNKS):
                xt = xpool.tile([C, w], BF, tag=f"x{cI}")
                nc.sync.dma_start(xt[:], x0T[:, col0:col0 + w])
                xs.append(xt)

            # ---- forward spectral transform for one chunk (piggybacked,
            # lagging the sweep by one chunk so the PE never stalls on x) ----
            def fwd_chunk(cI, ispec, parity):
                col0, w = CHUNKS[cI]
                ntl = w // 128
                ft = ps.tile([128, w], BF, tag="Br", padded_shape=[128, 512])
                for f in range(ntl):
                    nc.tensor.transpose(ft[:, f * 128:(f + 1) * 128],
                                        xs[cI][:, f * 128:(f + 1) * 128],
                                        ident_bf[:])
                xT = csb.tile([128, w], BF, tag="xT", padded_shape=[128, 512])
                if parity:
                    nc.vector.tensor_copy(xT[:], ft[:])
                else:
                    nc.scalar.activation(xT[:], ft[:], AF.Copy)
                t0 = col0 // 128
                for f in range(ntl):
                    t = t0 + f
                    nc.tensor.matmul(ispec[:], xT[:, f * 128:(f + 1) * 128],
                                     evm_c[:, t * 128:(t + 1) * 128],
                                     start=(t == 0), stop=(t == NT - 1))

            xspec = ps.tile([C, K], F32, tag="xspec")
            for cI in range(len(CHUNKS)):
                fwd_chunk(cI, xspec, parity=cI % 2)

            for i in range(nb):
                # ---- coefs multiply (pre-collective), AllReduce over pair ----
                STf_p = smalls.tile([C, K], F32, tag="STfp")
                nc.vector.tensor_mul(STf_p[:], xspec[:], coefsT_s[i][:])
                if collective:
                    cc_in = dram.tile([C, K], F32, tag="ccin")
                    cc_out = dram.tile([C, K], F32, tag="ccout")
                    nc.sync.dma_start(cc_in[:], STf_p[:])
                    nc.gpsimd.collective_compute(
                        "AllReduce", ALU.add,
                        replica_groups=PAIRS[:ncores // 2],
                        ins=[cc_in.opt()], outs=[cc_out.opt()])
                    STf = smalls.tile([C, K], F32, tag="STf")
                    nc.sync.dma_start(STf[:], cc_out[:])
                else:
                    STf = STf_p

                # ---- S (bf16), its A_re/A_im products, S@w0b ----
                STf_bf = smalls.tile([C, K], BF, tag="STfbf")
                nc.scalar.activation(STf_bf[:], STf[:], AF.Copy)
                S_ps = ps.tile([K, C], BF, tag="gX", padded_shape=[K, 512])
                nc.tensor.transpose(S_ps[:], STf_bf[:], ident_bf[:])
                S_bf = smalls.tile([K, C], BF, tag="Sbf")
                nc.vector.tensor_copy(S_bf[:], S_ps[:])
                Sre_ps = ps.tile([K, C], F32, tag="gY", padded_shape=[K, 512])
                nc.tensor.matmul(Sre_ps[:], STf_bf[:], Are_s[i][:],
                                 start=True, stop=True)
                Sre_bf = smalls.tile([K, C], BF, tag="Srebf")
                nc.scalar.activation(Sre_bf[:], Sre_ps[:], AF.Copy)
                Sim_ps = ps.tile([K, C], F32, tag="Br", padded_shape=[K, 512])
                nc.tensor.matmul(Sim_ps[:], STf_bf[:], Aim_s[i][:],
                                 start=True, stop=True)
                Sim_bf = smalls.tile([K, C], BF, tag="Simbf")
                nc.scalar.activation(Sim_bf[:], Sim_ps[:], AF.Copy)
                nSim_bf = smalls.tile([K, C], BF, tag="nSimbf")
                nc.vector.tensor_scalar_mul(nSim_bf[:], Sim_ps[:], -1.0)
                SW0b_ps = ps.tile([K, C], F32, tag="Bi", padded_shape=[K, 512])
                nc.tensor.matmul(SW0b_ps[:], STf_bf[:], w0b_s[i][:],
                                 start=True, stop=True)
                SW0b_bf = smalls.tile([K, C], BF, tag="SW0b")
                nc.scalar.activation(SW0b_bf[:], SW0b_ps[:], AF.Copy)

                if i < nb - 1:
                    xspec = ps.tile([C, K], F32, tag="xspec")

                # ---- fused per-node sweep ----
                for cI, (col0, w) in enumerate(CHUNKS):
                    csl = slice(col0, col0 + w)
                    gy_t = gyp.tile([K, w], BF, tag="gy", padded_shape=[K, 512])
                    nc.sync.dma_start(gy_t[:], gyT[:, csl])
                    gx_t = gx_c[:, csl]
                    ev_t = ev_c[:, csl]

                    # grad path: Br/Bi first (their copies gate m1/m2)
                    Br_ps = ps.tile([C, w], F32, tag="Br", padded_shape=[C, 512])
                    nc.tensor.matmul(Br_ps[:], Sre_bf[:], gx_t,
                                     start=True, stop=False)
                    nc.tensor.matmul(Br_ps[:], nSim_bf[:], gy_t[:],
                                     start=False, stop=True)
                    Bi_ps = ps.tile([C, w], F32, tag="Bi", padded_shape=[C, 512])
                    nc.tensor.matmul(Bi_ps[:], Sre_bf[:], gy_t[:],
                                     start=True, stop=False)
                    nc.tensor.matmul(Bi_ps[:], Sim_bf[:], gx_t,
                                     start=False, stop=True)
                    Br_sb = csb.tile([C, w], BF, tag="Br", padded_shape=[C, 512])
                    nc.scalar.activation(Br_sb[:], Br_ps[:], AF.Copy)
                    Bi_sb = csb.tile([C, w], BF, tag="Bi", padded_shape=[C, 512])
                    nc.scalar.activation(Bi_sb[:], Bi_ps[:], AF.Copy)

                    gX_ps = ps.tile([C, w], F32, tag="gX", padded_shape=[C, 512])
                    nc.tensor.matmul(gX_ps[:], S_bf[:], gx_t,
                                     start=True, stop=True)
                    m1 = csb.tile([C, w], BF, tag="m1", padded_shape=[C, 512])
                    nc.vector.tensor_mul(m1[:], gX_ps[:], Br_sb[:])
                    gY_ps = ps.tile([C, w], F32, tag="gY", padded_shape=[C, 512])
                    nc.tensor.matmul(gY_ps[:], S_bf[:], gy_t[:],
                                     start=True, stop=True)
                    m2 = csb.tile([C, w], BF, tag="m2", padded_shape=[C, 512])
                    nc.vector.tensor_mul(m2[:], gY_ps[:], Bi_sb[:])
                    a1 = csb.tile([C, w], BF, tag="a1", padded_shape=[C, 512])
                    nc.vector.tensor_add(a1[:], m1[:], m2[:])
                    gf = csb.tile([C, w], BF, tag="gf", padded_shape=[C, 512])
                    nc.scalar.activation(gf[:], a1[:], AF.Tanh)

                    # MLP path: x/ev partial sums early, gf term last
                    h0_ps = ps.tile([C, w], F32, tag="h0", padded_shape=[C, 512])
                    nc.tensor.matmul(h0_ps[:], w0a_s[i][:], xs[cI][:],
                                     start=True, stop=False)
                    nc.tensor.matmul(h0_ps[:], SW0b_bf[:], ev_t,
                                     start=False, stop=False)
                    nc.tensor.matmul(h0_ps[:], w0c_s[i][:], gf[:],
                                     start=False, stop=True)
                    h0_sb = csb.tile([C, w], BF, tag="h0", padded_shape=[C, 512])
                    nc.vector.tensor_scalar(h0_sb[:], h0_ps[:], b0_s[i][:],
                                            0.0, ALU.add, ALU.max)
                    h1_ps = ps.tile([C, w], F32, tag="h1", padded_shape=[C, 512])
                    nc.tensor.matmul(h1_ps[:], w1_s[i][:], h0_sb[:],
                                     start=True, stop=True)
                    h1_sb = csb.tile([C, w], BF, tag="h1", padded_shape=[C, 512])
                    nc.scalar.activation(h1_sb[:], h1_ps[:], AF.Relu,
                                         bias=b1_s[i][:])
                    h2_ps = ps.tile([C, w], F32, tag="h2", padded_shape=[C, 512])
                    nc.tensor.matmul(h2_ps[:], w2_s[i][:], h1_sb[:],
                                     start=True, stop=True)
                    # x += h2 + b2 (bf16 carry)
                    nc.vector.scalar_tensor_tensor(
                        out=xs[cI][:], in0=h2_ps[:], scalar=b2_s[i][:],
                        in1=xs[cI][:], op0=ALU.add, op1=ALU.add)

                    if i < nb - 1:
                        # piggybacked forward for the PREVIOUS chunk: its
                        # x tile was finalized a full chunk ago, so the PE
                        # transposes never wait on this chunk's tail
                        if cI >= 1:
                            fwd_chunk(cI - 1, xspec, parity=cI % 2)
                    else:
                        y_ps = ps.tile([3, w], F32, tag="h0",
                                       padded_shape=[3, 512])
                        nc.tensor.matmul(y_ps[:], wlast_s[:], xs[cI][:],
                                         start=True, stop=True)
                        y_sb = csb.tile([3, w], F32, tag="y",
                                        padded_shape=[3, 512], bufs=2)
                        nc.scalar.activation(y_sb[:], y_ps[:], AF.Identity,
                                             bias=blast_s[:])
                        nc.sync.dma_start(yT[:, csl], y_sb[:])
                if i < nb - 1:
                    fwd_chunk(len(CHUNKS) - 1, xspec, parity=0)

    nc.compile()
    return nc


_NC_CACHE = {}


def _get_nc():
    if "nc" not in _NC_CACHE:
        _NC_CACHE["nc"] = build_nc()
    return _NC_CACHE["nc"]


def kernel(**inputs):
    nc = _get_nc()
    in_maps = host_prep(inputs)
    res = run_bass_kernel_spmd(nc, in_maps, core_ids=list(range(NCORES)))
    out = np.empty((B, N, 3), np.float32)
    for b in range(B):
        for h in range(2):
            yT = res.results[2 * b + h]["yT"]
            out[b, h * NH:(h + 1) * NH] = yT[:, :NH].T
    return out


# revision 45
# speedup vs baseline: 2.5819x; 1.4432x over previous
"""DiffusionNet forward on 8 Trainium2 NeuronCores.

Strategy
--------
B=4 samples, 2 cores per sample, each core owns half the mesh nodes
(20000, zero-padded to 20096 = 157*128).  All cross-node coupling flows
through the K=128 spectral bottleneck:

  * SpMM is eliminated on-device: gX = G @ x_diffuse = (G @ evecs) @ S with
    S = coefs * x_spec, so host precomputes GXe = G @ evecs once per sample
    (exact associativity).
  * Static per-node operands (evm node-major, evecs K-major, GXe K-major)
    are cached in SBUF once; only GYe re-streams per block.  x is carried
    in bf16 channel-major SBUF tiles.
  * The forward spectral transform for block i+1 is piggybacked into the
    per-chunk sweep of block i (x^T tiles via DMA xbar transposes), so each
    block boundary is only the 64KB pairwise AllReduce plus a short
    S-matmul chain.
  * PSUM banks are assigned per-role (gX/gY/Br/Bi/h0/h1/h2/xspec) and
    SBUF working tiles triple-buffered so ~3 chunks pipeline across
    engines; elementwise work is split between DVE and Act.
"""

import sys
import numpy as np
import ml_dtypes

for _p in ("/opt/trn_rl_repo", "/root/.axon_site/_ro/trn_rl_repo"):
    if _p not in sys.path:
        sys.path.append(_p)

import concourse.bass as bass
import concourse.bacc as bacc
import concourse.tile as tile
import concourse.mybir as mybir
from concourse.bass_utils import run_bass_kernel_spmd
from concourse.masks import make_identity

BF = mybir.dt.bfloat16
F32 = mybir.dt.float32
AF = mybir.ActivationFunctionType
ALU = mybir.AluOpType

B, N, E, K = 4, 40000, 240000, 128
C = 128
NB = 4          # diffusion blocks
NCORES = 8
NH = N // 2     # nodes per core (half sample)
NHP = 20096     # padded nodes per core: 157 tiles * 128
NT = NHP // 128
# 39 full 512-wide chunks + one 128-wide tail chunk
CHUNKS = [(c * 512, 512) for c in range(39)] + [(39 * 512, 128)]
PAIRS = [[0, 1], [2, 3], [4, 5], [6, 7]]
USE_DMA_T = True   # xbar DMA transposes for the forward piggyback

bf16 = ml_dtypes.bfloat16


# ----------------------------------------------------------------- host side

def _spmm_mat(rows, cols, vals, M):
    """(COO [N,N] with given pattern) @ M, dense M [N,k]. Pure numpy."""
    out = np.zeros((N, M.shape[1]), np.float32)
    perm = np.argsort(rows, kind="stable")
    contrib = (vals[:, None] * M[cols]).astype(np.float32)[perm]
    rs = rows[perm]
    uniq, starts = np.unique(rs, return_index=True)
    out[uniq] = np.add.reduceat(contrib, starts, axis=0)
    return out


def host_prep(inputs, nhp=NHP, nb=NB):
    """Build the 8 per-core input dicts."""
    x_in = np.asarray(inputs["x_in"], np.float32)
    mass = np.asarray(inputs["mass"], np.float32)
    evals = np.asarray(inputs["evals"], np.float32)
    evecs = np.asarray(inputs["evecs"], np.float32)
    rows = np.asarray(inputs["rows"])
    cols = np.asarray(inputs["cols"])
    gX_vals = np.asarray(inputs["gradX_vals"], np.float32)
    gY_vals = np.asarray(inputs["gradY_vals"], np.float32)
    w_first = np.asarray(inputs["w_first"], np.float32)
    b_first = np.asarray(inputs["b_first"], np.float32)
    diff_time = np.asarray(inputs["diff_time"], np.float32)
    A_re = np.asarray(inputs["A_re"], np.float32)
    A_im = np.asarray(inputs["A_im"], np.float32)
    mlp_w0 = np.asarray(inputs["mlp_w0"], np.float32)
    w1 = np.asarray(inputs["mlp_w1"], np.float32)
    w2 = np.asarray(inputs["mlp_w2"], np.float32)
    b0 = np.asarray(inputs["mlp_b0"], np.float32)
    b1 = np.asarray(inputs["mlp_b1"], np.float32)
    b2 = np.asarray(inputs["mlp_b2"], np.float32)
    w_last = np.asarray(inputs["w_last"], np.float32)
    b_last = np.asarray(inputs["b_last"], np.float32)

    nh = NH

    shared = dict(
        Are=A_re[:nb].astype(bf16),
        Aim=A_im[:nb].astype(bf16),
        w0a=np.ascontiguousarray(mlp_w0[:nb, 0:C]).astype(bf16),
        w0b=np.ascontiguousarray(mlp_w0[:nb, C:2 * C]).astype(bf16),
        w0c=np.ascontiguousarray(mlp_w0[:nb, 2 * C:3 * C]).astype(bf16),
        w1=w1[:nb].astype(bf16),
        w2=w2[:nb].astype(bf16),
        b0=b0[:nb].reshape(nb, C, 1),
        b1=b1[:nb].reshape(nb, C, 1),
        b2=b2[:nb].reshape(nb, C, 1),
        wlast=w_last.astype(bf16),
        blast=b_last.reshape(3, 1),
    )

    in_maps = []
    for b in range(B):
        ev = evecs[b]
        evm_full = ev * mass[b][:, None]
        GXe = _spmm_mat(rows, cols, gX_vals[b], ev)
        GYe = _spmm_mat(rows, cols, gY_vals[b], ev)
        x0_full = x_in[b] @ w_first + b_first
        # coefsT[i][c,k] = exp(-evals[k] * diff_time[i][c])
        coefsT = np.exp(-evals[b][None, None, :]
                        * diff_time[:nb, :, None]).astype(np.float32)
        for h in range(2):
            sl = slice(h * nh, (h + 1) * nh)

            def padT(M):  # [nh, K] -> [K, nhp]
                out = np.zeros((M.shape[1], nhp), np.float32)
                out[:, :nh] = M[sl].T
                return out

            # per-tile node-major mass: massN[p, t] = mass[node t*128+p]
            massP = np.zeros((nhp,), np.float32)
            massP[:nh] = np.asarray(inputs["mass"], np.float32)[b][sl]
            massN = massP.reshape(NT, 128).T.copy()
            x0T = padT(x0_full)
            x0C = np.zeros((len(CHUNKS), C, 512), np.float32)
            for cJ, (col0, w) in enumerate(CHUNKS):
                x0C[cJ, :, :w] = x0T[:, col0:col0 + w]
            in_maps.append(dict(
                massN=massN,
                evT=padT(ev).astype(bf16),
                gxT=padT(GXe).astype(bf16),
                gyT=padT(GYe).astype(bf16),
                x0C=x0C.astype(bf16),
                coefsT=coefsT,
                **shared,
            ))
    return in_maps


# --------------------------------------------------------------- device side

def build_nc(nb=NB, ncores=NCORES, collective=True):
    nhp = NHP
    nc = bacc.Bacc("TRN2", target_bir_lowering=False, debug=False,
                   enable_asserts=True, num_devices=ncores)

    massN = nc.dram_tensor("massN", [128, NT], F32, kind="ExternalInput")
    evT = nc.dram_tensor("evT", [K, nhp], BF, kind="ExternalInput")
    gxT = nc.dram_tensor("gxT", [K, nhp], BF, kind="ExternalInput")
    gyT = nc.dram_tensor("gyT", [K, nhp], BF, kind="ExternalInput")
    x0C = nc.dram_tensor("x0C", [len(CHUNKS), C, 512], BF,
                         kind="ExternalInput")
    coefsT = nc.dram_tensor("coefsT", [nb, C, K], F32, kind="ExternalInput")
    Are = nc.dram_tensor("Are", [nb, C, C], BF, kind="ExternalInput")
    Aim = nc.dram_tensor("Aim", [nb, C, C], BF, kind="ExternalInput")
    w0a = nc.dram_tensor("w0a", [nb, C, C], BF, kind="ExternalInput")
    w0b = nc.dram_tensor("w0b", [nb, C, C], BF, kind="ExternalInput")
    w0c = nc.dram_tensor("w0c", [nb, C, C], BF, kind="ExternalInput")
    w1 = nc.dram_tensor("w1", [nb, C, C], BF, kind="ExternalInput")
    w2 = nc.dram_tensor("w2", [nb, C, C], BF, kind="ExternalInput")
    b0 = nc.dram_tensor("b0", [nb, C, 1], F32, kind="ExternalInput")
    b1 = nc.dram_tensor("b1", [nb, C, 1], F32, kind="ExternalInput")
    b2 = nc.dram_tensor("b2", [nb, C, 1], F32, kind="ExternalInput")
    wlast = nc.dram_tensor("wlast", [C, 3], BF, kind="ExternalInput")
    blast = nc.dram_tensor("blast", [3, 1], F32, kind="ExternalInput")
    yT = nc.dram_tensor("yT", [3, nhp], BF, kind="ExternalOutput")

    with tile.TileContext(nc) as tc:
        with (
            tc.tile_pool(name="consts", bufs=1) as consts,
            tc.tile_pool(name="xpool", bufs=1) as xpool,
            tc.tile_pool(name="gyp", bufs=8) as gyp,
            tc.tile_pool(name="smalls", bufs=1) as smalls,
            tc.tile_pool(name="csb", bufs=3) as csb,
            tc.tile_pool(name="ps", bufs=1, space="PSUM") as ps,
            tc.tile_pool(name="dram", bufs=2, space="DRAM") as dram,
        ):
            ident_bf = consts.tile([128, 128], BF, tag="identb")
            make_identity(nc, ident_bf[:])

            # warm-up collective: pays the gpsimd collective library load
            # and aligns the pair while the initial DMAs stream, so the
            # first real AllReduce doesn't eat ~20us of setup
            if collective:
                wu = consts.tile([1, 8], F32, tag="wu")
                nc.vector.memset(wu[:], 0.0)
                wu_in = dram.tile([1, 8], F32, tag="wuin")
                wu_out = dram.tile([1, 8], F32, tag="wuout")
                nc.sync.dma_start(wu_in[:], wu[:])
                nc.gpsimd.collective_compute(
                    "AllReduce", ALU.add,
                    replica_groups=PAIRS[:ncores // 2],
                    ins=[wu_in.opt()], outs=[wu_out.opt()])

            def cload(src, shape, dt, tag):
                t = consts.tile(shape, dt, tag=tag)
                nc.sync.dma_start(t[:], src)
                return t

            Are_s = [cload(Are[i], [C, C], BF, f"Are{i}") for i in range(nb)]
            Aim_s = [cload(Aim[i], [C, C], BF, f"Aim{i}") for i in range(nb)]
            coefsT_s = [cload(coefsT[i], [C, K], F32, f"cf{i}") for i in range(nb)]
            w0a_s = [cload(w0a[i], [C, C], BF, f"w0a{i}") for i in range(nb)]
            w0b_s = [cload(w0b[i], [C, C], BF, f"w0b{i}") for i in range(nb)]
            w0c_s = [cload(w0c[i], [C, C], BF, f"w0c{i}") for i in range(nb)]
            w1_s = [cload(w1[i], [C, C], BF, f"w1{i}") for i in range(nb)]
            w2_s = [cload(w2[i], [C, C], BF, f"w2{i}") for i in range(nb)]
            b0_s = [cload(b0[i], [C, 1], F32, f"b0{i}") for i in range(nb)]
            b1_s = [cload(b1[i], [C, 1], F32, f"b1{i}") for i in range(nb)]
            b2_s = [cload(b2[i], [C, 1], F32, f"b2{i}") for i in range(nb)]
            wlast_s = cload(wlast[:], [C, 3], BF, "wlast")
            blast_s = cload(blast[:], [3, 1], F32, "blast")

            # ---- SBUF caches for the static streams (stripe the DMAs so
            # they spread across queues) ----
            def cache(src, tag, nstripe=8):
                t = consts.tile([128, nhp], BF, tag=tag)
                stripe = nhp // nstripe
                for s in range(nstripe):
                    sl = slice(s * stripe, (s + 1) * stripe)
                    nc.sync.dma_start(t[:, sl], src[:, sl])
                return t

            # load order matters for startup latency: ev + x0 first,
            # interleaved so fwd0 can start while loads stream.  evm is
            # BUILT on-device from ev (transpose + per-node mass scale) to
            # halve the HBM bytes on the critical startup path.  The
            # node-major x0 copy (x0N, consumed only by fwd0) streams
            # through the small xT ring; gx loads right behind.
            massN_s = cload(massN[:], [128, NT], F32, "massN")
            ev_c = consts.tile([128, nhp], BF, tag="evc")
            gx_c = consts.tile([128, nhp], BF, tag="gxc")
            evm_c = consts.tile([128, nhp], BF, tag="evmc")
            xs = [xpool.tile([C, w], BF, tag=f"x{cI}", name=f"x{cI}")
                  for cI, (col0, w) in enumerate(CHUNKS)]
            estripe = nhp // 8
            for s in range(8):
                sl = slice(s * estripe, (s + 1) * estripe)
                nc.sync.dma_start(ev_c[:, sl], evT[:, sl])

            # fused evm build + fwd0, paced by the ev/x0 stream arrival.
            # x0 is transposed on the PE (like the sweep piggyback); the
            # ft tiles rotate through the otherwise-idle gXY/BrBi banks so
            # consecutive chunks never serialize on a PSUM slot.
            HT = ["h0", "h1", "h2"]
            FT = ["gXY", "BrBi"]
            xspec = ps.tile([C, K], F32, tag="xspec")
            for cI, (col0, w) in enumerate(CHUNKS):
                nc.sync.dma_start(xs[cI][:], x0C[cI, :, 0:w])
                for f in range(w // 128):
                    t = col0 // 128 + f
                    tsl = slice(t * 128, (t + 1) * 128)
                    tp = ps.tile([128, 128], BF, tag=HT[t % 3],
                                 padded_shape=[128, 512])
                    nc.tensor.transpose(tp[:], ev_c[:, tsl], ident_bf[:])
                    if t % 2:
                        nc.vector.tensor_scalar_mul(evm_c[:, tsl], tp[:],
                                                    massN_s[:, t:t + 1])
                    else:
                        nc.scalar.activation(evm_c[:, tsl], tp[:],
                                             AF.Identity,
                                             scale=massN_s[:, t:t + 1])
                ft = ps.tile([128, w], BF, tag=FT[cI % 2],
                             padded_shape=[128, 512])
                for f in range(w // 128):
                    nc.tensor.transpose(ft[:, f * 128:(f + 1) * 128],
                                        xs[cI][:, f * 128:(f + 1) * 128],
                                        ident_bf[:])
                xT0 = csb.tile([128, w], BF, tag="xT", padded_shape=[128, 512])
                if cI % 2:
                    nc.vector.tensor_copy(xT0[:], ft[:])
                else:
                    nc.scalar.activation(xT0[:], ft[:], AF.Copy)
                for f in range(w // 128):
                    t = col0 // 128 + f
                    nc.tensor.matmul(xspec[:], xT0[:, f * 128:(f + 1) * 128],
                                     evm_c[:, t * 128:(t + 1) * 128],
                                     start=(t == 0), stop=(t == NT - 1))

            # ---- forward spectral transform for one chunk (piggybacked,
            # lagging the sweep by one chunk so the PE never stalls on x) ----
            def fwd_chunk(cI, ispec, parity):
                col0, w = CHUNKS[cI]
                ntl = w // 128
                ft = ps.tile([128, w], BF, tag="h1", padded_shape=[128, 512])
                for f in range(ntl):
                    nc.tensor.transpose(ft[:, f * 128:(f + 1) * 128],
                                        xs[cI][:, f * 128:(f + 1) * 128],
                                        ident_bf[:])
                xT = csb.tile([128, w], BF, tag="xT", padded_shape=[128, 512])
                if parity:
                    nc.vector.tensor_copy(xT[:], ft[:])
                else:
                    nc.scalar.activation(xT[:], ft[:], AF.Copy)
                t0 = col0 // 128
                for f in range(ntl):
                    t = t0 + f
                    nc.tensor.matmul(ispec[:], xT[:, f * 128:(f + 1) * 128],
                                     evm_c[:, t * 128:(t + 1) * 128],
                                     start=(t == 0), stop=(t == NT - 1))

            for s in range(8):
                sl = slice(s * estripe, (s + 1) * estripe)
                nc.sync.dma_start(gx_c[:, sl], gxT[:, sl])

            for i in range(nb):
                # ---- coefs multiply (pre-collective), AllReduce over pair ----
                STf_p = smalls.tile([C, K], F32, tag="STfp")
                nc.vector.tensor_mul(STf_p[:], xspec[:], coefsT_s[i][:])
                if collective:
                    cc_in = dram.tile([C, K], F32, tag="ccin")
                    cc_out = dram.tile([C, K], F32, tag="ccout")
                    nc.sync.dma_start(cc_in[:], STf_p[:])
                    nc.gpsimd.collective_compute(
                        "AllReduce", ALU.add,
                        replica_groups=PAIRS[:ncores // 2],
                        ins=[cc_in.opt()], outs=[cc_out.opt()])
                    STf = smalls.tile([C, K], F32, tag="STf")
                    nc.sync.dma_start(STf[:], cc_out[:])
                else:
                    STf = STf_p

                # ---- S (bf16), its A_re/A_im products, S@w0b ----
                STf_bf = smalls.tile([C, K], BF, tag="STfbf")
                nc.scalar.activation(STf_bf[:], STf[:], AF.Copy)
                S_ps = ps.tile([K, C], BF, tag="gXY", padded_shape=[K, 1024])
                nc.tensor.transpose(S_ps[:], STf_bf[:], ident_bf[:])
                S_bf = smalls.tile([K, C], BF, tag="Sbf")
                nc.vector.tensor_copy(S_bf[:], S_ps[:])
                Sre_ps = ps.tile([K, C], F32, tag="BrBi",
                                 padded_shape=[K, 1024])
                nc.tensor.matmul(Sre_ps[:], STf_bf[:], Are_s[i][:],
                                 start=True, stop=True)
                Sre_bf = smalls.tile([K, C], BF, tag="Srebf")
                nc.scalar.activation(Sre_bf[:], Sre_ps[:], AF.Copy)
                Sim_ps = ps.tile([K, C], F32, tag="h0", padded_shape=[K, 512])
                nc.tensor.matmul(Sim_ps[:], STf_bf[:], Aim_s[i][:],
                                 start=True, stop=True)
                Sim_bf = smalls.tile([K, C], BF, tag="Simbf")
                nc.scalar.activation(Sim_bf[:], Sim_ps[:], AF.Copy)
                nSim_bf = smalls.tile([K, C], BF, tag="nSimbf")
                nc.vector.tensor_scalar_mul(nSim_bf[:], Sim_ps[:], -1.0)
                SW0b_ps = ps.tile([K, C], F32, tag="h2", padded_shape=[K, 512])
                nc.tensor.matmul(SW0b_ps[:], STf_bf[:], w0b_s[i][:],
                                 start=True, stop=True)
                SW0b_bf = smalls.tile([K, C], BF, tag="SW0b")
                nc.scalar.activation(SW0b_bf[:], SW0b_ps[:], AF.Copy)

                if i < nb - 1:
                    xspec = ps.tile([C, K], F32, tag="xspec")

                # ---- fused per-node sweep ----
                for cI, (col0, w) in enumerate(CHUNKS):
                    csl = slice(col0, col0 + w)
                    gy_t = gyp.tile([K, w], BF, tag="gy", padded_shape=[K, 512])
                    nc.sync.dma_start(gy_t[:], gyT[:, csl])
                    gx_t = gx_c[:, csl]
                    ev_t = ev_c[:, csl]

                    # grad path: Br/Bi packed in one 2-bank PSUM tile so the
                    # copy and the elementwise multiply are single wide ops
                    BrBi = ps.tile([C, 2 * w], F32, tag="BrBi",
                                   padded_shape=[C, 1024])
                    nc.tensor.matmul(BrBi[:, 0:w], Sre_bf[:], gx_t,
                                     start=True, stop=False)
                    nc.tensor.matmul(BrBi[:, 0:w], nSim_bf[:], gy_t[:],
                                     start=False, stop=True)
                    nc.tensor.matmul(BrBi[:, w:2 * w], Sre_bf[:], gy_t[:],
                                     start=True, stop=False)
                    nc.tensor.matmul(BrBi[:, w:2 * w], Sim_bf[:], gx_t,
                                     start=False, stop=True)
                    BrBi_sb = csb.tile([C, 2 * w], BF, tag="BrBi",
                                       padded_shape=[C, 1024])
                    nc.scalar.activation(BrBi_sb[:], BrBi[:], AF.Copy)

                    gXY = ps.tile([C, 2 * w], F32, tag="gXY",
                                  padded_shape=[C, 1024])
                    nc.tensor.matmul(gXY[:, 0:w], S_bf[:], gx_t,
                                     start=True, stop=True)
                    nc.tensor.matmul(gXY[:, w:2 * w], S_bf[:], gy_t[:],
                                     start=True, stop=True)
                    m12 = csb.tile([C, 2 * w], BF, tag="m12",
                                   padded_shape=[C, 1024])
                    nc.vector.tensor_mul(m12[:], gXY[:], BrBi_sb[:])
                    a1 = csb.tile([C, w], BF, tag="a1", padded_shape=[C, 512])
                    nc.gpsimd.tensor_add(a1[:], m12[:, 0:w], m12[:, w:2 * w])
                    gf = csb.tile([C, w], BF, tag="gf", padded_shape=[C, 512])
                    nc.scalar.activation(gf[:], a1[:], AF.Tanh)

                    # MLP path: x/ev partial sums early, gf term last
                    h0_ps = ps.tile([C, w], F32, tag="h0", padded_shape=[C, 512])
                    nc.tensor.matmul(h0_ps[:], w0a_s[i][:], xs[cI][:],
                                     start=True, stop=False)
                    nc.tensor.matmul(h0_ps[:], SW0b_bf[:], ev_t,
                                     start=False, stop=False)
                    nc.tensor.matmul(h0_ps[:], w0c_s[i][:], gf[:],
                                     start=False, stop=True)
                    h0_sb = csb.tile([C, w], BF, tag="h0", padded_shape=[C, 512])
                    nc.vector.tensor_scalar(h0_sb[:], h0_ps[:], b0_s[i][:],
                                            0.0, ALU.add, ALU.max)
                    h1_ps = ps.tile([C, w], F32, tag="h1", padded_shape=[C, 512])
                    nc.tensor.matmul(h1_ps[:], w1_s[i][:], h0_sb[:],
                                     start=True, stop=True)
                    h1_sb = csb.tile([C, w], BF, tag="h1", padded_shape=[C, 512])
                    nc.scalar.activation(h1_sb[:], h1_ps[:], AF.Relu,
                                         bias=b1_s[i][:])
                    h2_ps = ps.tile([C, w], F32, tag="h2", padded_shape=[C, 512])
                    nc.tensor.matmul(h2_ps[:], w2_s[i][:], h1_sb[:],
                                     start=True, stop=True)
                    # x += h2 + b2 (bf16 carry)
                    nc.vector.scalar_tensor_tensor(
                        out=xs[cI][:], in0=h2_ps[:], scalar=b2_s[i][:],
                        in1=xs[cI][:], op0=ALU.add, op1=ALU.add)

                    if i < nb - 1:
                        # piggybacked forward for the PREVIOUS chunk: its
                        # x tile was finalized a full chunk ago, so the PE
                        # transposes never wait on this chunk's tail
                        if cI >= 1:
                            fwd_chunk(cI - 1, xspec, parity=cI % 2)
                    else:
                        # b_last is added host-side during the gather; the
                        # xspec bank is idle in the last block — use it so
                        # the y path never couples to the h-chain banks
                        y_ps = ps.tile([3, w], F32, tag="xspec",
                                       padded_shape=[3, 512])
                        nc.tensor.matmul(y_ps[:], wlast_s[:], xs[cI][:],
                                         start=True, stop=True)
                        y_sb = csb.tile([3, w], BF, tag="y",
                                        padded_shape=[3, 512], bufs=2)
                        if cI % 2:
                            nc.vector.tensor_copy(y_sb[:], y_ps[:])
                        else:
                            nc.scalar.activation(y_sb[:], y_ps[:], AF.Copy)
                        nc.sync.dma_start(yT[:, csl], y_sb[:])
                if i < nb - 1:
                    fwd_chunk(len(CHUNKS) - 1, xspec, parity=0)

    nc.compile()
    return nc


_NC_CACHE = {}


def _get_nc():
    if "nc" not in _NC_CACHE:
        _NC_CACHE["nc"] = build_nc()
    return _NC_CACHE["nc"]


def kernel(**inputs):
    nc = _get_nc()
    in_maps = host_prep(inputs)
    res = run_bass_kernel_spmd(nc, in_maps, core_ids=list(range(NCORES)))
    b_last = np.asarray(inputs["b_last"], np.float32)
    out = np.empty((B, N, 3), np.float32)
    for b in range(B):
        for h in range(2):
            yT = np.asarray(res.results[2 * b + h]["yT"], np.float32)
            out[b, h * NH:(h + 1) * NH] = yT[:, :NH].T + b_last
    return out
